# revision 1
# baseline (speedup 1.0000x reference)
"""2-layer 8-head GAT forward, distributed over 8 Trainium2 NeuronCores.

Strategy (graph data parallelism, per sharding hint):
  - Edges sorted by destination; dst nodes blocked by 128; 80 blocks sharded
    10-per-core. All index preprocessing is host-side (integers only).
  - Per layer each core holds two DRAM tables (bf16):
      fat  row n = [ h(n) (256) | alpha_src(n) (8) | pad ]  (768 B)
      tail row n = [ alpha_dst(n) (8) | pad ]               (256 B)
    h = x @ W with the attention vectors folded in (W_ext = [W|W@As|W@Ad]).
  - Edge phase per dst block:
      * batched dma_gather of fat rows by src id (1024-idx chunks, 4 queues)
      * alpha_dst for the block's 128 dst nodes: one direct DMA (no gather)
      * per 128-edge tile, host-supplied selection matrices (bf16 0/1):
          Sel_de[d,e] expands alpha_dst to edges via a PE matmul (N=8)
          SelT[e,d] accumulates rhs = [ex*h | ex] into PSUM (N=264)
        with ex = exp(leaky(s+d)) = max(exp(t), exp(0.2 t)) exactly.
      * epilogue divides by the summed ex, adds bias (+ ELU after layer 1).
  - Layer-1 activations AllGather'd per dst block (bf16, overlapped with the
    remaining edge work); each core rebuilds the layer-2 tables from them.
Output: each core writes its 1280 dst rows f32; host concatenates and trims.
"""

import math
import os
import sys

for _p in ("/opt/trn_rl_repo", "/root/.axon_site/_ro/trn_rl_repo"):
    if os.path.isdir(_p) and _p not in sys.path:
        sys.path.append(_p)

import numpy as np

from concourse import bass, bacc, mybir
import concourse.tile as tile
from concourse.masks import make_identity
from concourse.bass_utils import run_bass_kernel_spmd

F32 = mybir.dt.float32
BF16 = mybir.dt.bfloat16
I16 = mybir.dt.int16
AF = mybir.ActivationFunctionType
OP = mybir.AluOpType
P = 128


class Cfg:
    def __init__(self, n_nodes=10000, n_edges=320000, hid=256, heads=8, n_cores=8):
        self.N = n_nodes
        self.E = n_edges
        self.HID = hid
        self.H = heads
        self.C = hid // heads
        self.NC = n_cores
        self.NP = -(-n_nodes // (P * n_cores)) * (P * n_cores)
        self.NT = self.NP // P
        self.BPC = self.NT // n_cores
        self.NW = hid + 2 * heads        # table matmul width (h | s | d)
        self.NW2 = hid + heads           # edge matmul rhs width (ex*h | ex)
        self.FAT = hid + 128             # fat row elems (bf16): h | s | pad
        self.TAIL = 128                  # tail row elems (bf16): d | pad
        self.ag_group = 1
        assert self.FAT * 2 % 256 == 0 and self.TAIL * 2 % 256 == 0


# --------------------------------------------------------------------------
# Host preprocessing (indices / selection masks / weight folding)
# --------------------------------------------------------------------------
def _wrap16(idx, T):
    w = idx.reshape(-1, 16).T.astype(np.int16)
    return np.tile(w, (8, 1))


def _a_expand(a, cfg):
    A = np.zeros((cfg.H, cfg.C, cfg.H), np.float32)
    for h in range(cfg.H):
        A[h, :, h] = a[h]
    return A.reshape(cfg.HID, cfg.H)


def preprocess(cfg, x, edges_idx, W1, a_src1, a_dst1, b1, W2, a_src2, a_dst2, b2):
    import ml_dtypes

    bfd = ml_dtypes.bfloat16

    src = np.asarray(edges_idx[0], np.int64)
    dst = np.asarray(edges_idx[1], np.int64)
    order = np.argsort(dst, kind="stable")
    src_s, dst_s = src[order], dst[order]
    blk = dst_s // P
    counts = np.bincount(blk, minlength=cfg.NT)
    T = max(1, int(-(-counts.max() // P)))
    starts = np.concatenate([[0], np.cumsum(counts)])
    EPB = T * P

    isrc = np.zeros((cfg.NC, cfg.BPC, P, 8 * T), np.int16)
    iblk = np.zeros((cfg.NC, cfg.BPC, P, 8), np.int16)
    selT = np.zeros((cfg.NC, cfg.BPC, T, P, P), bfd)   # [e, d] per tile
    selD = np.zeros((cfg.NC, cfg.BPC, T, P, P), bfd)   # [d, e] per tile
    iota = np.arange(P)
    for gb in range(cfg.NT):
        c, b = gb // cfg.BPC, gb % cfg.BPC
        s0, s1 = starts[gb], starts[gb + 1]
        n = s1 - s0
        a_src = np.zeros(EPB, np.int64)
        a_loc = np.full(EPB, -1, np.int64)
        a_src[:n] = src_s[s0:s1]
        a_loc[:n] = dst_s[s0:s1] - gb * P
        isrc[c, b] = _wrap16(a_src, T)
        iblk[c, b] = _wrap16(np.arange(gb * P, (gb + 1) * P, dtype=np.int64), 1)
        loc_t = a_loc.reshape(T, P)                   # [t, e]
        st = (loc_t[:, :, None] == iota[None, None, :])  # [t, e, d]
        selT[c, b] = st.astype(bfd)
        selD[c, b] = np.swapaxes(st, 1, 2).astype(bfd)

    W1e = np.concatenate(
        [W1, W1 @ _a_expand(a_src1, cfg), W1 @ _a_expand(a_dst1, cfg)], axis=1
    ).astype(np.float32)
    W2e = np.concatenate(
        [W2, W2 @ _a_expand(a_src2, cfg), W2 @ _a_expand(a_dst2, cfg)], axis=1
    ).astype(np.float32)

    xT = np.zeros((cfg.HID, cfg.NP), np.float32)
    xT[:, : cfg.N] = np.asarray(x, np.float32).T
    b1b = np.broadcast_to(np.asarray(b1, np.float32), (P, cfg.HID)).copy()
    b2b = np.broadcast_to(np.asarray(b2, np.float32), (P, cfg.HID)).copy()

    shared = {
        "xT": xT.astype(bfd), "w1e": W1e.astype(bfd), "w2e": W2e.astype(bfd),
        "b1b": b1b, "b2b": b2b,
    }
    in_maps = [
        dict(shared, isrc=isrc[c], iblk=iblk[c], selt=selT[c], seld=selD[c])
        for c in range(cfg.NC)
    ]
    return in_maps, T


# --------------------------------------------------------------------------
# Device program
# --------------------------------------------------------------------------
NSTAGE = 4


def _make_stage_bufs(nc, fp, cfg):
    fats, tails = [], []
    for i in range(NSTAGE):
        f = fp.tile([P, cfg.FAT], BF16, tag=f"fatbuf{i}")
        nc.gpsimd.memset(f[:, cfg.HID + cfg.H : cfg.FAT], 0.0)
        fats.append(f)
        t = fp.tile([P, cfg.TAIL], BF16, tag=f"tailbuf{i}")
        nc.gpsimd.memset(t[:, cfg.H : cfg.TAIL], 0.0)
        tails.append(t)
    return fats, tails


def _table_from_lhsT(nc, pp, lhsT0, lhsT1, wa, wb, fat, tl, f_dst, t_dst, cfg):
    ps = pp.tile([P, cfg.NW], F32, tag="tps")
    nc.tensor.matmul(ps[:], lhsT0, wa[:], start=True, stop=False)
    nc.tensor.matmul(ps[:], lhsT1, wb[:], start=False, stop=True)
    nc.vector.tensor_copy(fat[:, 0 : cfg.HID + cfg.H], ps[:, 0 : cfg.HID + cfg.H])
    nc.vector.tensor_copy(tl[:, 0 : cfg.H], ps[:, cfg.HID + cfg.H : cfg.NW])
    nc.scalar.dma_start(f_dst, fat[:])
    nc.scalar.dma_start(t_dst, tl[:])


def _edge_phase(nc, tc, cfg, T, tabf, tabt, isrc_e, iblk_e, selt_e, seld_e,
                bias_t, layer, zsh=None, zfull_blocks=None, out_e=None):
    HID, H, C, NW2, FAT, TAIL, BPC = (
        cfg.HID, cfg.H, cfg.C, cfg.NW2, cfg.FAT, cfg.TAIL, cfg.BPC,
    )
    CH = 8  # 1024-idx gather chunks (HW limit)
    qn = [0]
    with (
        tc.tile_pool(name=f"ge{layer}", bufs=3) as gp,
        tc.tile_pool(name=f"ix{layer}", bufs=2) as ip,
        tc.tile_pool(name=f"sl{layer}", bufs=2) as slp,
        tc.tile_pool(name=f"wk{layer}", bufs=4) as wp,
        tc.tile_pool(name=f"eo{layer}", bufs=2) as op_,
        tc.tile_pool(name=f"eps{layer}", bufs=2, space="PSUM") as pp,
        tc.tile_pool(name=f"dps{layer}", bufs=4, space="PSUM") as dpp,
    ):
        for b in range(BPC):
            ist = ip.tile([P, 8 * T], I16, tag="isrc")
            nc.sync.dma_start(ist[:], isrc_e[b])
            # selection matrices, both layouts: [P, T, P] (partition = e / d)
            slt = slp.tile([P, T, P], BF16, tag="slt")
            nc.sync.dma_start(
                slt[:], selt_e[b].rearrange("t e d -> e t d")
            )
            sld = slp.tile([P, T, P], BF16, tag="sld")
            nc.sync.dma_start(
                sld[:], seld_e[b].rearrange("t d e -> d t e")
            )
            # alpha_dst rows for this block: one tiny 128-idx gather
            ibt = ip.tile([P, 8], I16, tag="iblk")
            nc.sync.dma_start(ibt[:], iblk_e[b])
            adb = ip.tile([P, 1, TAIL], BF16, tag="adb")
            nc.gpsimd.dma_gather(
                out_ap=adb[:], in_ap=tabt[:, :], idxs_ap=ibt[:],
                num_idxs=P, num_idxs_reg=P, elem_size=TAIL,
                queue_num=qn[0] % 4,
            )
            qn[0] += 1

            gA = gp.tile([P, T, FAT], BF16, tag="gA")
            for c0 in range(0, T, CH):
                cw = min(CH, T - c0)
                nc.gpsimd.dma_gather(
                    out_ap=gA[:, c0 : c0 + cw, :], in_ap=tabf[:, :],
                    idxs_ap=ist[:, c0 * 8 : (c0 + cw) * 8],
                    num_idxs=P * cw, num_idxs_reg=P * cw, elem_size=FAT,
                    queue_num=qn[0] % 4,
                )
                qn[0] += 1

            ps = pp.tile([P, NW2], F32, tag="eps")
            for t0 in range(0, T, 2):
                tw = min(2, T - t0)
                # expand alpha_dst to edges: PE matmul, N=8 per tile
                dx = dpp.tile([P, 2 * H], F32, tag="dx")
                for j in range(tw):
                    nc.tensor.matmul(
                        dx[:, j * H : (j + 1) * H], sld[:, t0 + j, :],
                        adb[:, 0, 0:H], start=True, stop=True,
                    )
                ts2 = wp.tile([P, 2, H], F32, tag="ts")
                nc.vector.tensor_tensor(
                    ts2[:, 0:tw, :], gA[:, t0 : t0 + tw, HID : HID + H],
                    dx[:, 0 : tw * H].rearrange("p (j h) -> p j h", j=tw),
                    op=OP.add,
                )
                rhs2 = wp.tile([P, 2, NW2], BF16, tag="rhs")
                ea2 = wp.tile([P, 2, H], F32, tag="ea")
                # exp(leaky(t)) == max(exp(t), exp(0.2 t)) exactly
                nc.scalar.activation(ea2[:, 0:tw, :], ts2[:, 0:tw, :], AF.Exp)
                nc.scalar.activation(
                    rhs2[:, 0:tw, HID : HID + H], ts2[:, 0:tw, :], AF.Exp,
                    scale=0.2,
                )
                nc.vector.tensor_tensor(
                    rhs2[:, 0:tw, HID : HID + H], ea2[:, 0:tw, :],
                    rhs2[:, 0:tw, HID : HID + H], op=OP.max,
                )
                for j in range(tw):
                    t = t0 + j
                    nc.vector.tensor_tensor(
                        rhs2[:, j, 0:HID].rearrange("p (h c) -> p h c", h=H),
                        gA[:, t, 0:HID].rearrange("p (h c) -> p h c", h=H),
                        rhs2[:, j, HID : HID + H].to_broadcast([P, H, C]),
                        op=OP.mult,
                    )
                    nc.tensor.matmul(
                        ps[:], slt[:, t, :], rhs2[:, j, :],
                        start=(t == 0), stop=(t == T - 1),
                    )
            # epilogue
            den = wp.tile([P, H], F32, tag="den")
            nc.vector.tensor_scalar_add(den[:], ps[:, HID : HID + H], 1e-16)
            rec = wp.tile([P, H], F32, tag="rec")
            nc.vector.reciprocal(rec[:], den[:])
            ot = op_.tile([P, HID], F32, tag="ot")
            nc.vector.tensor_tensor(
                ot[:].rearrange("p (h c) -> p h c", h=H),
                ps[:, 0:HID].rearrange("p (h c) -> p h c", h=H),
                rec[:].to_broadcast([P, H, C]),
                op=OP.mult,
            )
            nc.vector.tensor_tensor(ot[:], ot[:], bias_t[:], op=OP.add)
            if layer == 1:
                # ELU(x) = relu(x) + exp(min(x,0)) - 1 -> bf16 z
                r_ = op_.tile([P, HID], F32, tag="relu")
                nc.scalar.activation(r_[:], ot[:], AF.Relu)
                m_ = op_.tile([P, HID], F32, tag="mneg")
                nc.vector.tensor_tensor(m_[:], ot[:], r_[:], op=OP.subtract)
                nc.scalar.activation(m_[:], m_[:], AF.Exp)
                nc.vector.tensor_scalar_add(m_[:], m_[:], -1.0)
                zt = op_.tile([P, HID], BF16, tag="zt")
                nc.vector.tensor_tensor(zt[:], r_[:], m_[:], op=OP.add)
                nc.sync.dma_start(zsh[b * P : (b + 1) * P, :], zt[:])
                # overlap the exchange with the remaining blocks: one
                # AllGather per pair of dst blocks
                gs = cfg.ag_group
                if b % gs == gs - 1:
                    g = b // gs
                    nc.gpsimd.collective_compute(
                        "AllGather", OP.bypass,
                        replica_groups=[list(range(cfg.NC))],
                        ins=[zsh[(b - gs + 1) * P : (b + 1) * P, :]],
                        outs=[zfull_blocks[g][:]],
                    )
            else:
                nc.sync.dma_start(out_e[b * P : (b + 1) * P, :], ot[:])


def build_program(cfg, T, stages="full"):
    nc = bacc.Bacc(num_swdge_queues=4)
    HID, NW, NP, NT, BPC = cfg.HID, cfg.NW, cfg.NP, cfg.NT, cfg.BPC

    xT_e = nc.declare_dram_parameter("xT", [HID, NP], BF16, isOutput=False)
    w1_e = nc.declare_dram_parameter("w1e", [HID, NW], BF16, isOutput=False)
    w2_e = nc.declare_dram_parameter("w2e", [HID, NW], BF16, isOutput=False)
    b1_e = nc.declare_dram_parameter("b1b", [P, HID], F32, isOutput=False)
    b2_e = nc.declare_dram_parameter("b2b", [P, HID], F32, isOutput=False)
    isrc_e = nc.declare_dram_parameter("isrc", [BPC, P, 8 * T], I16, isOutput=False)
    iblk_e = nc.declare_dram_parameter("iblk", [BPC, P, 8], I16, isOutput=False)
    selt_e = nc.declare_dram_parameter("selt", [BPC, T, P, P], BF16, isOutput=False)
    seld_e = nc.declare_dram_parameter("seld", [BPC, T, P, P], BF16, isOutput=False)
    out_e = nc.declare_dram_parameter("out", [BPC * P, HID], F32, isOutput=True)

    tabf1 = nc.dram_tensor("tabf1", [NP, cfg.FAT], BF16)
    tabf2 = nc.dram_tensor("tabf2", [NP, cfg.FAT], BF16)
    tabt1 = nc.dram_tensor("tabt1", [NP, cfg.TAIL], BF16)
    tabt2 = nc.dram_tensor("tabt2", [NP, cfg.TAIL], BF16)
    zsh = nc.dram_tensor("zsh", [BPC * P, HID], BF16)
    # one gathered tensor per group of dst blocks: the layer-2 table build
    # can start per-group as collectives land
    GS = cfg.ag_group
    NG = BPC // GS
    zfb = [
        nc.dram_tensor(f"zfull{g}", [cfg.NC, GS * P, HID], BF16,
                       addr_space="Shared")
        for g in range(NG)
    ]

    with tile.TileContext(nc) as tc:
        with tc.tile_pool(name="const", bufs=1) as cp:
            w1a = cp.tile([P, NW], BF16)
            nc.sync.dma_start(w1a[:], w1_e[0:P, :])
            w1b = cp.tile([P, NW], BF16)
            nc.sync.dma_start(w1b[:], w1_e[P : 2 * P, :])
            w2a = cp.tile([P, NW], BF16)
            nc.sync.dma_start(w2a[:], w2_e[0:P, :])
            w2b = cp.tile([P, NW], BF16)
            nc.sync.dma_start(w2b[:], w2_e[P : 2 * P, :])
            b1t = cp.tile([P, HID], F32)
            nc.sync.dma_start(b1t[:], b1_e[:, :])
            b2t = cp.tile([P, HID], F32)
            nc.sync.dma_start(b2t[:], b2_e[:, :])
            idn = cp.tile([P, P], BF16)
            make_identity(nc, idn[:])

            # ---- layer-1 tables (full, redundant per core) ----
            PAN = 10
            with (
                tc.tile_pool(name="s1", bufs=2) as sp,
                tc.tile_pool(name="fp1", bufs=1) as fp,
                tc.tile_pool(name="ps1", bufs=3, space="PSUM") as pp,
            ):
                fats, tails = _make_stage_bufs(nc, fp, cfg)
                for pan in range(-(-NT // PAN)):
                    j0, j1 = pan * PAN, min(NT, (pan + 1) * PAN)
                    w = (j1 - j0) * P
                    xp0 = sp.tile([P, PAN * P], BF16, tag="xp0")
                    nc.sync.dma_start(xp0[:, :w], xT_e[0:P, j0 * P : j1 * P])
                    xp1 = sp.tile([P, PAN * P], BF16, tag="xp1")
                    nc.sync.dma_start(xp1[:, :w], xT_e[P : 2 * P, j0 * P : j1 * P])
                    for j in range(j0, j1):
                        o = (j - j0) * P
                        _table_from_lhsT(
                            nc, pp, xp0[:, o : o + P], xp1[:, o : o + P],
                            w1a, w1b, fats[j % NSTAGE], tails[j % NSTAGE],
                            tabf1[j * P : (j + 1) * P, :],
                            tabt1[j * P : (j + 1) * P, :], cfg,
                        )

            if stages in ("t1e1", "full"):
                _edge_phase(
                    nc, tc, cfg, T, tabf1, tabt1, isrc_e, iblk_e, selt_e,
                    seld_e, b1t, layer=1, zsh=zsh, zfull_blocks=zfb,
                )

            if stages == "t1e1":
                with tc.tile_pool(name="dbg", bufs=2) as dp:
                    for b in range(BPC):
                        dt_ = dp.tile([P, HID], BF16, tag="dbg")
                        nc.sync.dma_start(dt_[:], zsh[b * P : (b + 1) * P, :])
                        dt2 = dp.tile([P, HID], F32, tag="dbg2")
                        nc.vector.tensor_copy(dt2[:], dt_[:])
                        nc.sync.dma_start(out_e[b * P : (b + 1) * P, :], dt2[:])

            if stages == "full":
                # ---- layer-2 tables from gathered z (per-group ready) ----
                # zT panels come straight from a DMA transpose of each
                # group's gathered rows (bf16 HWDGE xbar path).
                with (
                    tc.tile_pool(name="s2", bufs=3) as sp,
                    tc.tile_pool(name="fp2", bufs=1) as fp,
                    tc.tile_pool(name="ps2", bufs=4, space="PSUM") as pp,
                ):
                    fats, tails = _make_stage_bufs(nc, fp, cfg)
                    for g in range(NG):
                        rows = cfg.NC * GS * P
                        zp0 = sp.tile([P, rows], BF16, tag="zp0")
                        nc.sync.dma_start(
                            zp0[:],
                            zfb[g][:].rearrange("r p h -> (r p) h")[:, 0:P],
                            transpose=True,
                        )
                        zp1 = sp.tile([P, rows], BF16, tag="zp1")
                        nc.sync.dma_start(
                            zp1[:],
                            zfb[g][:].rearrange("r p h -> (r p) h")[:, P : 2 * P],
                            transpose=True,
                        )
                        for r in range(cfg.NC):
                            for l2 in range(GS):
                                j = r * BPC + g * GS + l2
                                o = r * GS * P + l2 * P
                                _table_from_lhsT(
                                    nc, pp, zp0[:, o : o + P], zp1[:, o : o + P],
                                    w2a, w2b,
                                    fats[j % NSTAGE], tails[j % NSTAGE],
                                    tabf2[j * P : (j + 1) * P, :],
                                    tabt2[j * P : (j + 1) * P, :], cfg,
                                )

                _edge_phase(
                    nc, tc, cfg, T, tabf2, tabt2, isrc_e, iblk_e, selt_e,
                    seld_e, b2t, layer=2, out_e=out_e,
                )
    nc.finalize()
    return nc


# --------------------------------------------------------------------------
# Entry point
# --------------------------------------------------------------------------
def run_gat(inputs, cfg=None, trace=False):
    cfg = cfg or Cfg()
    in_maps, T = preprocess(cfg, **inputs)
    nc = build_program(cfg, T)
    res = run_bass_kernel_spmd(nc, in_maps, list(range(cfg.NC)), trace=trace)
    out = np.concatenate([res.results[c]["out"] for c in range(cfg.NC)], axis=0)
    return out[: cfg.N], res


def kernel(**inputs) -> np.ndarray:
    out, _ = run_gat(inputs)
    return np.ascontiguousarray(out, dtype=np.float32)



# revision 5
# speedup vs baseline: 1.3484x; 1.3484x over previous
"""2-layer 8-head GAT forward, distributed over 8 Trainium2 NeuronCores.

Strategy (graph data parallelism, per sharding hint):
  - Edges sorted by destination; dst nodes blocked by 128; 80 blocks sharded
    10-per-core. All index preprocessing is host-side (integers only).
  - Per layer each core holds ONE DRAM fat table (bf16):
      row n = [ h(n) (256) | alpha_src(n) (8) | alpha_dst(n) (8) | pad ] (768 B)
    built as h = x @ W_ext with attention vectors folded in
    (W_ext = [W | W@As | W@Ad]).
  - Edge phase per dst block (software-pipelined prep/front/back stages):
      * batched dma_gather of fat rows by src id (1024-idx chunks, 4 queues)
      * alpha_dst rows for the block's own 128 dsts: one 256B-elem gather
      * selection matrices generated ON-CHIP from tiny index vectors:
          selT[e,t,d] = (loc[e,t] == d)            one DVE is_equal per block
          sld[d,e]    = (start[d] <= e < end[d])   two DVE ops per block
        (edges are sorted by dst, so per-dst edges form contiguous runs)
      * dx = sld_t @ alpha_dst (PE, N=8); ts = s + dx;
        ex = exp(leaky(ts)) = max(exp(ts), exp(0.2 ts)) exactly;
        rhs = [ex*h | ex] (bf16, one fused 4D DVE multiply per block);
        PSUM accumulation over tiles via selT matmuls (N=264).
      * epilogue divides by summed ex, adds bias (+ ELU after layer 1).
  - Layer-2 tables are built LOCALLY per core from its own z blocks
    (PE transpose of z + table matmul, no DRAM round-trip for z), then
    exchanged with two AllGathers (halves, to overlap the first with the
    tail of the layer-1 edge phase). No per-block collectives.
Output: each core writes its 1280 dst rows f32; host concatenates and trims.
"""

import os
import sys

for _p in ("/opt/trn_rl_repo", "/root/.axon_site/_ro/trn_rl_repo"):
    if os.path.isdir(_p) and _p not in sys.path:
        sys.path.append(_p)

import numpy as np

from concourse import bacc, mybir
import concourse.tile as tile
from concourse.masks import make_identity
from concourse.bass_utils import run_bass_kernel_spmd

F32 = mybir.dt.float32
BF16 = mybir.dt.bfloat16
I16 = mybir.dt.int16
AF = mybir.ActivationFunctionType
OP = mybir.AluOpType
P = 128


class Cfg:
    def __init__(self, n_nodes=10000, n_edges=320000, hid=256, heads=8, n_cores=8):
        self.N = n_nodes
        self.E = n_edges
        self.HID = hid
        self.H = heads
        self.C = hid // heads
        self.NC = n_cores
        self.NP = -(-n_nodes // (P * n_cores)) * (P * n_cores)
        self.NT = self.NP // P
        self.BPC = self.NT // n_cores
        self.NW = hid + 2 * heads        # table matmul width (h | s | d)
        self.NW2 = hid + heads           # edge matmul rhs width (ex*h | ex)
        self.FAT = hid + 128             # fat row elems (bf16): h | s | d | pad
        self.HB = (self.BPC // 2) * P    # rows per AllGather half
        assert self.FAT * 2 % 256 == 0


# --------------------------------------------------------------------------
# Host preprocessing (indices / weight folding)
# --------------------------------------------------------------------------
def _wrap16(idx):
    w = idx.reshape(-1, 16).T.astype(np.int16)
    return np.tile(w, (8, 1))


def _a_expand(a, cfg):
    A = np.zeros((cfg.H, cfg.C, cfg.H), np.float32)
    for h in range(cfg.H):
        A[h, :, h] = a[h]
    return A.reshape(cfg.HID, cfg.H)


def _remap2(n, cfg):
    """Row index of node n in the two-half AllGather'd layer-2 table."""
    r, l = np.divmod(n, cfg.BPC * P)
    half, lh = np.divmod(l, cfg.HB)
    return half * (cfg.NC * cfg.HB) + r * cfg.HB + lh


def preprocess(cfg, x, edges_idx, W1, a_src1, a_dst1, b1, W2, a_src2, a_dst2, b2):
    import ml_dtypes

    bfd = ml_dtypes.bfloat16

    src = np.asarray(edges_idx[0], np.int64)
    dst = np.asarray(edges_idx[1], np.int64)
    order = np.argsort(dst, kind="stable")
    src_s, dst_s = src[order], dst[order]
    blk = dst_s // P
    counts = np.bincount(blk, minlength=cfg.NT)
    T = max(1, int(-(-counts.max() // P)))
    starts = np.concatenate([[0], np.cumsum(counts)])
    EPB = T * P

    isrc = np.zeros((cfg.NC, cfg.BPC, P, 8 * T), np.int16)
    isrc2 = np.zeros((cfg.NC, cfg.BPC, P, 8 * T), np.int16)
    iblk = np.zeros((cfg.NC, cfg.BPC, P, 8), np.int16)
    iblk2 = np.zeros((cfg.NC, cfg.BPC, P, 8), np.int16)
    loce = np.zeros((cfg.NC, cfg.BPC, P, T), np.int16)
    sede = np.zeros((cfg.NC, cfg.BPC, P, 2), np.float32)
    for gb in range(cfg.NT):
        c, b = gb // cfg.BPC, gb % cfg.BPC
        s0, s1 = starts[gb], starts[gb + 1]
        n = s1 - s0
        a_src = np.zeros(EPB, np.int64)
        a_loc = np.full(EPB, -1, np.int64)
        a_src[:n] = src_s[s0:s1]
        a_loc[:n] = dst_s[s0:s1] - gb * P
        isrc[c, b] = _wrap16(a_src)
        isrc2[c, b] = _wrap16(_remap2(a_src, cfg))
        own = np.arange(gb * P, (gb + 1) * P, dtype=np.int64)
        iblk[c, b] = _wrap16(own)
        iblk2[c, b] = _wrap16(_remap2(own, cfg))
        loce[c, b] = a_loc.reshape(T, P).T            # [e, t]
        # per-dst contiguous run bounds within the block's sorted edges
        cnt_d = np.bincount(a_loc[:n], minlength=P)
        end_d = np.cumsum(cnt_d)
        sede[c, b, :, 0] = (end_d - cnt_d).astype(np.float32)
        sede[c, b, :, 1] = end_d.astype(np.float32)

    W1e = np.concatenate(
        [W1, W1 @ _a_expand(a_src1, cfg), W1 @ _a_expand(a_dst1, cfg)], axis=1
    ).astype(np.float32)
    W2e = np.concatenate(
        [W2, W2 @ _a_expand(a_src2, cfg), W2 @ _a_expand(a_dst2, cfg)], axis=1
    ).astype(np.float32)

    xT = np.zeros((cfg.HID, cfg.NP), np.float32)
    xT[:, : cfg.N] = np.asarray(x, np.float32).T
    b1b = np.broadcast_to(np.asarray(b1, np.float32), (P, cfg.HID)).copy()
    b2b = np.broadcast_to(np.asarray(b2, np.float32), (P, cfg.HID)).copy()

    shared = {
        "xT": xT.astype(bfd), "w1e": W1e.astype(bfd), "w2e": W2e.astype(bfd),
        "b1b": b1b, "b2b": b2b,
    }
    in_maps = [
        dict(shared, isrc=isrc[c], isrc2=isrc2[c], iblk=iblk[c], iblk2=iblk2[c],
             loce=loce[c], sede=sede[c])
        for c in range(cfg.NC)
    ]
    return in_maps, T


# --------------------------------------------------------------------------
# Device program
# --------------------------------------------------------------------------
def _edge_phase(nc, tc, cfg, T, cn, layer, tabf, isrc_e, iblk_e, loc_e, se_e,
                fat2own=None, tabf2sh=None, out_e=None):
    """Edge phase for one layer, software-pipelined over this core's blocks.

    layer==1: epilogue applies ELU, builds the layer-2 table rows for the
    block locally (PE transpose + matmul) and stages them for AllGather.
    layer==2: epilogue writes the final f32 output rows.
    """
    HID, H, C, NW2, FAT, BPC = cfg.HID, cfg.H, cfg.C, cfg.NW2, cfg.FAT, cfg.BPC
    CH = 8  # 1024-idx gather chunks (HW limit)
    qn = [0]
    bias_t = cn["b1t"] if layer == 1 else cn["b2t"]
    st = {}  # per-block live tiles

    with (
        tc.tile_pool(name=f"ge{layer}", bufs=2) as gp,
        tc.tile_pool(name=f"ix{layer}", bufs=3) as ip,
        tc.tile_pool(name=f"sl{layer}", bufs=2) as slp,
        tc.tile_pool(name=f"wk{layer}", bufs=2) as wp,
        tc.tile_pool(name=f"eo{layer}", bufs=2) as op_,
        tc.tile_pool(name=f"eps{layer}", bufs=2, space="PSUM") as pp,
        tc.tile_pool(name=f"dps{layer}", bufs=2, space="PSUM") as dpp,
        tc.tile_pool(name=f"l2ps{layer}", bufs=1, space="PSUM") as l2pp,
        tc.tile_pool(name=f"l2sb{layer}", bufs=3) as l2sp,
    ):
        def prep(b):
            s = st[b] = {}
            ist = ip.tile([P, 8 * T], I16, tag="isrc")
            nc.sync.dma_start(ist[:], isrc_e[b])
            loc = ip.tile([P, T], I16, tag="loc")
            nc.sync.dma_start(loc[:], loc_e[b])
            se = ip.tile([P, 2], F32, tag="se")
            nc.sync.dma_start(se[:], se_e[b])
            ibt = ip.tile([P, 8], I16, tag="iblk")
            nc.sync.dma_start(ibt[:], iblk_e[b])
            # on-chip selection masks
            slt = slp.tile([P, T, P], BF16, tag="slt")
            nc.vector.tensor_tensor(
                slt[:], loc[:].to_broadcast([P, T, P]), cn["iota_td"][:],
                op=OP.is_equal,
            )
            lt = slp.tile([P, T * P], BF16, tag="lt")
            nc.vector.tensor_scalar(
                lt[:], cn["iota_e"][:], se[:, 1:2], None, op0=OP.is_lt,
            )
            sld = slp.tile([P, T * P], BF16, tag="sld")
            nc.vector.scalar_tensor_tensor(
                sld[:], cn["iota_e"][:], se[:, 0:1], lt[:],
                op0=OP.is_ge, op1=OP.mult,
            )
            s["slt"], s["sld"] = slt, sld
            # alpha_dst for the block's 128 dsts: 256B-elem gather of the
            # tail half of the fat rows ([s | d | pad])
            adb = ip.tile([P, 1, P], BF16, tag="adb")
            nc.gpsimd.dma_gather(
                out_ap=adb[:], in_ap=tabf[:, HID:FAT], idxs_ap=ibt[:],
                num_idxs=P, num_idxs_reg=P, elem_size=P, elem_step=FAT,
                queue_num=qn[0] % 4,
            )
            qn[0] += 1
            s["adb"] = adb
            # fat-row gather by src id
            gA = gp.tile([P, T, FAT], BF16, tag="gA")
            for c0 in range(0, T, CH):
                cw = min(CH, T - c0)
                nc.gpsimd.dma_gather(
                    out_ap=gA[:, c0 : c0 + cw, :], in_ap=tabf[:, :],
                    idxs_ap=ist[:, c0 * 8 : (c0 + cw) * 8],
                    num_idxs=P * cw, num_idxs_reg=P * cw, elem_size=FAT,
                    queue_num=qn[0] % 4,
                )
                qn[0] += 1
            s["gA"] = gA

        def front(b):
            s = st[b]
            gA, slt, sld, adb = s["gA"], s["slt"], s["sld"], s["adb"]
            # expand alpha_dst to edges: one PE matmul (N=8) per tile
            dx = dpp.tile([P, T * H], F32, tag="dx")
            for t in range(T):
                nc.tensor.matmul(
                    dx[:, t * H : (t + 1) * H], sld[:, t * P : (t + 1) * P],
                    adb[:, 0, H : 2 * H], start=True, stop=True,
                )
            ts = wp.tile([P, T, H], F32, tag="ts")
            nc.vector.tensor_tensor(
                ts[:], gA[:, :, HID : HID + H],
                dx[:].rearrange("p (t h) -> p t h", t=T), op=OP.add,
            )
            ea = wp.tile([P, T, H], F32, tag="ea")
            nc.scalar.activation(ea[:], ts[:], AF.Exp)
            rhs = gp.tile([P, T, NW2], BF16, tag="rhs")
            exs = rhs[:, :, HID : HID + H]
            nc.scalar.activation(exs, ts[:], AF.Exp, scale=0.2)
            # exp(leaky(t)) == max(exp(t), exp(0.2 t)) exactly
            nc.vector.tensor_tensor(exs, ea[:], exs, op=OP.max)
            nc.vector.tensor_tensor(
                rhs[:, :, 0:HID].rearrange("p t (h c) -> p t h c", h=H),
                gA[:, :, 0:HID].rearrange("p t (h c) -> p t h c", h=H),
                exs.rearrange("p t (h o) -> p t h o", h=H).to_broadcast(
                    [P, T, H, C]
                ),
                op=OP.mult,
            )
            s["rhs"] = rhs

        def back(b):
            s = st.pop(b)
            slt, rhs = s["slt"], s["rhs"]
            ps = pp.tile([P, NW2], F32, tag="eps")
            for t in range(T):
                nc.tensor.matmul(
                    ps[:], slt[:, t, :], rhs[:, t, :],
                    start=(t == 0), stop=(t == T - 1),
                )
            den = op_.tile([P, H], F32, tag="den")
            nc.vector.tensor_scalar_add(den[:], ps[:, HID : HID + H], 1e-16)
            rec = op_.tile([P, H], F32, tag="rec")
            nc.vector.reciprocal(rec[:], den[:])
            ot = op_.tile([P, HID], F32, tag="ot")
            nc.vector.tensor_tensor(
                ot[:].rearrange("p (h c) -> p h c", h=H),
                ps[:, 0:HID].rearrange("p (h c) -> p h c", h=H),
                rec[:].to_broadcast([P, H, C]),
                op=OP.mult,
            )
            nc.vector.tensor_tensor(ot[:], ot[:], bias_t[:], op=OP.add)
            if layer == 1:
                # ELU(x) = relu(x) + exp(min(x,0)) - 1 -> bf16 z
                r_ = op_.tile([P, HID], F32, tag="relu")
                nc.scalar.activation(r_[:], ot[:], AF.Relu)
                m_ = op_.tile([P, HID], F32, tag="mneg")
                nc.vector.tensor_tensor(m_[:], ot[:], r_[:], op=OP.subtract)
                nc.scalar.activation(m_[:], m_[:], AF.Exp)
                nc.vector.tensor_scalar_add(m_[:], m_[:], -1.0)
                zt = op_.tile([P, HID], BF16, tag="zt")
                nc.vector.tensor_tensor(zt[:], r_[:], m_[:], op=OP.add)
                # build this block's layer-2 table rows locally:
                # transpose z on the PE, then fold-in W2_ext
                psT = l2pp.tile([P, 2 * P], BF16, tag="psT")
                nc.tensor.transpose(psT[:, 0:P], zt[:, 0:P], cn["idn"][:])
                nc.tensor.transpose(psT[:, P : 2 * P], zt[:, P : 2 * P],
                                    cn["idn"][:])
                zT = l2sp.tile([P, 2 * P], BF16, tag="zT")
                nc.vector.tensor_copy(zT[:], psT[:])
                ps2 = l2pp.tile([P, cfg.NW], F32, tag="ps2")
                nc.tensor.matmul(ps2[:], zT[:, 0:P], cn["w2a"][:],
                                 start=True, stop=False)
                nc.tensor.matmul(ps2[:], zT[:, P : 2 * P], cn["w2b"][:],
                                 start=False, stop=True)
                f2 = l2sp.tile([P, FAT], BF16, tag="f2")
                nc.vector.tensor_copy(f2[:, 0 : cfg.NW], ps2[:])
                nc.scalar.dma_start(fat2own[b * P : (b + 1) * P, :], f2[:])
            else:
                nc.sync.dma_start(out_e[b * P : (b + 1) * P, :], ot[:])

        def maybe_ag(b):
            if layer != 1:
                return
            HBR = cfg.HB
            if b == BPC // 2 - 1:
                nc.gpsimd.collective_compute(
                    "AllGather", OP.bypass,
                    replica_groups=[list(range(cfg.NC))],
                    ins=[fat2own[0:HBR, :]],
                    outs=[tabf2sh[0 : cfg.NC * HBR, :]],
                )
            elif b == BPC - 1:
                nc.gpsimd.collective_compute(
                    "AllGather", OP.bypass,
                    replica_groups=[list(range(cfg.NC))],
                    ins=[fat2own[HBR : 2 * HBR, :]],
                    outs=[tabf2sh[cfg.NC * HBR : 2 * cfg.NC * HBR, :]],
                )

        prep(0)
        for i in range(BPC):
            if i + 1 < BPC:
                prep(i + 1)
            front(i)
            if i >= 1:
                back(i - 1)
                maybe_ag(i - 1)
        back(BPC - 1)
        maybe_ag(BPC - 1)


def build_program(cfg, T):
    nc = bacc.Bacc(num_swdge_queues=4)
    HID, NW, NP, NT, BPC = cfg.HID, cfg.NW, cfg.NP, cfg.NT, cfg.BPC

    xT_e = nc.declare_dram_parameter("xT", [HID, NP], BF16, isOutput=False)
    w1_e = nc.declare_dram_parameter("w1e", [HID, NW], BF16, isOutput=False)
    w2_e = nc.declare_dram_parameter("w2e", [HID, NW], BF16, isOutput=False)
    b1_e = nc.declare_dram_parameter("b1b", [P, HID], F32, isOutput=False)
    b2_e = nc.declare_dram_parameter("b2b", [P, HID], F32, isOutput=False)
    isrc_e = nc.declare_dram_parameter("isrc", [BPC, P, 8 * T], I16, isOutput=False)
    isrc2_e = nc.declare_dram_parameter("isrc2", [BPC, P, 8 * T], I16, isOutput=False)
    iblk_e = nc.declare_dram_parameter("iblk", [BPC, P, 8], I16, isOutput=False)
    iblk2_e = nc.declare_dram_parameter("iblk2", [BPC, P, 8], I16, isOutput=False)
    loc_e = nc.declare_dram_parameter("loce", [BPC, P, T], I16, isOutput=False)
    se_e = nc.declare_dram_parameter("sede", [BPC, P, 2], F32, isOutput=False)
    out_e = nc.declare_dram_parameter("out", [BPC * P, HID], F32, isOutput=True)

    tabf1 = nc.dram_tensor("tabf1", [NP, cfg.FAT], BF16)
    fat2own = nc.dram_tensor("fat2own", [BPC * P, cfg.FAT], BF16)
    tabf2sh = nc.dram_tensor("tabf2sh", [NP, cfg.FAT], BF16, addr_space="Shared")

    with tile.TileContext(nc) as tc:
        with tc.tile_pool(name="const", bufs=1) as cp:
            cn = {}
            for nm, src in (("w1a", w1_e), ("w1b", w1_e), ("w2a", w2_e),
                            ("w2b", w2_e)):
                t = cp.tile([P, NW], BF16, tag=nm)
                lo = 0 if nm.endswith("a") else P
                nc.sync.dma_start(t[:], src[lo : lo + P, :])
                cn[nm] = t
            b1t = cp.tile([P, HID], F32)
            nc.sync.dma_start(b1t[:], b1_e[:, :])
            cn["b1t"] = b1t
            b2t = cp.tile([P, HID], F32)
            nc.sync.dma_start(b2t[:], b2_e[:, :])
            cn["b2t"] = b2t
            idn = cp.tile([P, P], BF16)
            make_identity(nc, idn[:])
            cn["idn"] = idn
            iota_td = cp.tile([P, T, P], I16)
            nc.gpsimd.iota(iota_td[:], pattern=[[0, T], [1, P]],
                           channel_multiplier=0)
            cn["iota_td"] = iota_td
            iota_e = cp.tile([P, T * P], I16)
            nc.gpsimd.iota(iota_e[:], pattern=[[1, T * P]],
                           channel_multiplier=0)
            cn["iota_e"] = iota_e

            # ---- layer-1 tables (full, redundant per core) ----
            PAN = 10
            with (
                tc.tile_pool(name="s1", bufs=2) as sp,
                tc.tile_pool(name="fp1", bufs=4) as fp,
                tc.tile_pool(name="ps1", bufs=4, space="PSUM") as pp,
            ):
                for pan in range(-(-NT // PAN)):
                    j0, j1 = pan * PAN, min(NT, (pan + 1) * PAN)
                    w = (j1 - j0) * P
                    xp0 = sp.tile([P, PAN * P], BF16, tag="xp0")
                    nc.sync.dma_start(xp0[:, :w], xT_e[0:P, j0 * P : j1 * P])
                    xp1 = sp.tile([P, PAN * P], BF16, tag="xp1")
                    nc.sync.dma_start(xp1[:, :w], xT_e[P : 2 * P, j0 * P : j1 * P])
                    for j in range(j0, j1):
                        o = (j - j0) * P
                        ps = pp.tile([P, NW], F32, tag="tps")
                        nc.tensor.matmul(ps[:], xp0[:, o : o + P], cn["w1a"][:],
                                         start=True, stop=False)
                        nc.tensor.matmul(ps[:], xp1[:, o : o + P], cn["w1b"][:],
                                         start=False, stop=True)
                        fat = fp.tile([P, cfg.FAT], BF16, tag="fat")
                        nc.vector.tensor_copy(fat[:, 0:NW], ps[:])
                        nc.scalar.dma_start(tabf1[j * P : (j + 1) * P, :], fat[:])

            _edge_phase(nc, tc, cfg, T, cn, 1, tabf1, isrc_e, iblk_e, loc_e,
                        se_e, fat2own=fat2own, tabf2sh=tabf2sh)
            _edge_phase(nc, tc, cfg, T, cn, 2, tabf2sh, isrc2_e, iblk2_e,
                        loc_e, se_e, out_e=out_e)
    nc.finalize()
    return nc


# --------------------------------------------------------------------------
# Entry point
# --------------------------------------------------------------------------
def run_gat(inputs, cfg=None, trace=False):
    cfg = cfg or Cfg()
    in_maps, T = preprocess(cfg, **inputs)
    nc = build_program(cfg, T)
    res = run_bass_kernel_spmd(nc, in_maps, list(range(cfg.NC)), trace=trace)
    out = np.concatenate([res.results[c]["out"] for c in range(cfg.NC)], axis=0)
    return out[: cfg.N], res


def kernel(**inputs) -> np.ndarray:
    out, _ = run_gat(inputs)
    return np.ascontiguousarray(out, dtype=np.float32)


# revision 8
# speedup vs baseline: 1.4567x; 1.0804x over previous
"""2-layer 8-head GAT forward, distributed over 8 Trainium2 NeuronCores.

Strategy (graph data parallelism, per sharding hint):
  - Edges sorted by destination; dst nodes blocked by 128; 80 blocks sharded
    10-per-core. All index preprocessing is host-side (integers only).
  - Per layer each core holds ONE DRAM fat table (bf16):
      row n = [ h(n) (256) | alpha_src(n) (8) | alpha_dst(n) (8) | pad ] (768 B)
    built as h = x @ W_ext with attention vectors folded in
    (W_ext = [W | W@As | W@Ad]).
  - Edge phase per dst block (software-pipelined prep/front/back stages):
      * batched dma_gather of fat rows by src id (4 SWDGE queues)
      * alpha_dst rows for the block's own 128 dsts: one 256B-elem gather
      * selection matrices generated ON-CHIP from tiny index vectors, in
        DVE fast-mode friendly layouts (packed 16-bit last dims):
          selT[e,d,t] = (loc[e,t] == d)         one 2x DVE is_equal per block
          sga[d,e] = (e >= start[d])            4x DVE tensor_scalar
          sgb[d,e] = (e >= end[d])              4x DVE tensor_scalar
        (sld = sga - sgb; the subtraction is folded into the PE by
        accumulating sga@adb + sgb@(-adb))
      * ts = s + dx; ex = exp(lrelu(ts)) via two chained ACT ops;
        rhs = [ex*h | ex] (bf16); PSUM accumulation via selT matmuls (N=264).
      * epilogue divides by summed ex, adds bias (+ ELU after layer 1).
  - Layer-2 tables are built LOCALLY per core from its own z blocks
    (PE transpose of z + table matmul, no DRAM round-trip for z), then
    exchanged with five pipelined AllGathers (2 blocks each) overlapping
    the layer-1 edge phase tail. No per-block collectives.
Output: each core writes its 1280 dst rows f32; host concatenates and trims.
"""

import os
import sys

for _p in ("/opt/trn_rl_repo", "/root/.axon_site/_ro/trn_rl_repo"):
    if os.path.isdir(_p) and _p not in sys.path:
        sys.path.append(_p)

import numpy as np

from concourse import bacc, mybir
import concourse.tile as tile
from concourse.masks import make_identity
from concourse.bass_utils import run_bass_kernel_spmd

F32 = mybir.dt.float32
BF16 = mybir.dt.bfloat16
I16 = mybir.dt.int16
AF = mybir.ActivationFunctionType
OP = mybir.AluOpType
P = 128
NSPLIT = 5  # AllGather pipeline depth (blocks-per-core per collective = 2)


class Cfg:
    def __init__(self, n_nodes=10000, n_edges=320000, hid=256, heads=8, n_cores=8):
        self.N = n_nodes
        self.E = n_edges
        self.HID = hid
        self.H = heads
        self.C = hid // heads
        self.NC = n_cores
        self.NP = -(-n_nodes // (P * n_cores)) * (P * n_cores)
        self.NT = self.NP // P
        self.BPC = self.NT // n_cores
        self.NW = hid + 2 * heads        # table matmul width (h | s | d)
        self.NW2 = hid + heads           # edge matmul rhs width (ex*h | ex)
        self.FAT = hid + 128             # fat row elems (bf16): h | s | d | pad
        self.HB = (self.BPC // NSPLIT) * P   # rows per AllGather slice
        assert self.BPC % NSPLIT == 0
        assert self.FAT * 2 % 256 == 0


# --------------------------------------------------------------------------
# Host preprocessing (indices / weight folding)
# --------------------------------------------------------------------------
def _wrap16(idx):
    w = idx.reshape(-1, 16).T.astype(np.int16)
    return np.tile(w, (8, 1))


def _a_expand(a, cfg):
    A = np.zeros((cfg.H, cfg.C, cfg.H), np.float32)
    for h in range(cfg.H):
        A[h, :, h] = a[h]
    return A.reshape(cfg.HID, cfg.H)


def _remap2(n, cfg):
    """Row index of node n in the NSPLIT-sliced AllGather'd layer-2 table."""
    r, l = np.divmod(n, cfg.BPC * P)
    part, lh = np.divmod(l, cfg.HB)
    return part * (cfg.NC * cfg.HB) + r * cfg.HB + lh


def preprocess(cfg, x, edges_idx, W1, a_src1, a_dst1, b1, W2, a_src2, a_dst2, b2):
    import ml_dtypes

    bfd = ml_dtypes.bfloat16

    src = np.asarray(edges_idx[0], np.int64)
    dst = np.asarray(edges_idx[1], np.int64)
    order = np.argsort(dst, kind="stable")
    src_s, dst_s = src[order], dst[order]
    blk = dst_s // P
    counts = np.bincount(blk, minlength=cfg.NT)
    T = max(1, int(-(-counts.max() // P)))
    starts = np.concatenate([[0], np.cumsum(counts)])
    EPB = T * P

    isrc = np.zeros((cfg.NC, cfg.BPC, P, 8 * T), np.int16)
    isrc2 = np.zeros((cfg.NC, cfg.BPC, P, 8 * T), np.int16)
    iblk = np.zeros((cfg.NC, cfg.BPC, P, 8), np.int16)
    iblk2 = np.zeros((cfg.NC, cfg.BPC, P, 8), np.int16)
    loce = np.zeros((cfg.NC, cfg.BPC, P, T), np.int16)
    sede = np.zeros((cfg.NC, cfg.BPC, P, 2), np.float32)
    for gb in range(cfg.NT):
        c, b = gb // cfg.BPC, gb % cfg.BPC
        s0, s1 = starts[gb], starts[gb + 1]
        n = s1 - s0
        a_src = np.zeros(EPB, np.int64)
        a_loc = np.full(EPB, -1, np.int64)
        a_src[:n] = src_s[s0:s1]
        a_loc[:n] = dst_s[s0:s1] - gb * P
        isrc[c, b] = _wrap16(a_src)
        isrc2[c, b] = _wrap16(_remap2(a_src, cfg))
        own = np.arange(gb * P, (gb + 1) * P, dtype=np.int64)
        iblk[c, b] = _wrap16(own)
        iblk2[c, b] = _wrap16(_remap2(own, cfg))
        loce[c, b] = a_loc.reshape(T, P).T            # [e, t]
        # per-dst contiguous run bounds within the block's sorted edges
        cnt_d = np.bincount(a_loc[:n], minlength=P)
        end_d = np.cumsum(cnt_d)
        sede[c, b, :, 0] = (end_d - cnt_d).astype(np.float32)
        sede[c, b, :, 1] = end_d.astype(np.float32)

    W1e = np.concatenate(
        [W1, W1 @ _a_expand(a_src1, cfg), W1 @ _a_expand(a_dst1, cfg)], axis=1
    ).astype(np.float32)
    W2e = np.concatenate(
        [W2, W2 @ _a_expand(a_src2, cfg), W2 @ _a_expand(a_dst2, cfg)], axis=1
    ).astype(np.float32)

    xT = np.zeros((cfg.HID, cfg.NP), np.float32)
    xT[:, : cfg.N] = np.asarray(x, np.float32).T
    b1b = np.broadcast_to(np.asarray(b1, np.float32), (P, cfg.HID)).copy()
    b2b = np.broadcast_to(np.asarray(b2, np.float32), (P, cfg.HID)).copy()

    shared = {
        "xT": xT.astype(bfd), "w1e": W1e.astype(bfd), "w2e": W2e.astype(bfd),
        "b1b": b1b, "b2b": b2b,
    }
    in_maps = [
        dict(shared, isrc=isrc[c], isrc2=isrc2[c], iblk=iblk[c], iblk2=iblk2[c],
             loce=loce[c], sede=sede[c])
        for c in range(cfg.NC)
    ]
    return in_maps, T


# --------------------------------------------------------------------------
# Device program
# --------------------------------------------------------------------------
def _edge_phase(nc, tc, cfg, T, cn, layer, tabf, isrc_e, iblk_e, loc_e, se_e,
                fat2own=None, tabf2sh=None, out_e=None):
    """Edge phase for one layer, software-pipelined over this core's blocks.

    layer==1: epilogue applies ELU, builds the layer-2 table rows for the
    block locally (PE transpose + matmul) and stages them for AllGather.
    layer==2: epilogue writes the final f32 output rows.
    """
    HID, H, C, NW2, FAT, BPC = cfg.HID, cfg.H, cfg.C, cfg.NW2, cfg.FAT, cfg.BPC
    CH = 8  # 1024-idx gather chunks (HW limit)
    qn = [0]
    bias_t = cn["b1t"] if layer == 1 else cn["b2t"]
    st = {}  # per-block live tiles

    with (
        tc.tile_pool(name=f"ge{layer}", bufs=2) as gp,
        tc.tile_pool(name=f"ix{layer}", bufs=3) as ip,
        tc.tile_pool(name=f"sl{layer}", bufs=2) as slp,
        tc.tile_pool(name=f"wk{layer}", bufs=2) as wp,
        tc.tile_pool(name=f"eo{layer}", bufs=2) as op_,
        tc.tile_pool(name=f"eps{layer}", bufs=2, space="PSUM") as pp,
        tc.tile_pool(name=f"dps{layer}", bufs=2, space="PSUM") as dpp,
        tc.tile_pool(name=f"l2ps{layer}", bufs=1, space="PSUM") as l2pp,
        tc.tile_pool(name=f"l2sb{layer}", bufs=3) as l2sp,
    ):
        def prep(b):
            s = st[b] = {}
            ist = ip.tile([P, 8 * T], I16, tag="isrc")
            nc.sync.dma_start(ist[:], isrc_e[b])
            loc = ip.tile([P, T], I16, tag="loc")
            nc.sync.dma_start(loc[:], loc_e[b])
            se = ip.tile([P, 2], F32, tag="se")
            nc.sync.dma_start(se[:], se_e[b])
            ibt = ip.tile([P, 8], I16, tag="iblk")
            nc.sync.dma_start(ibt[:], iblk_e[b])
            # on-chip selection masks (all operands packed 16-bit last dim)
            slt = slp.tile([P, P, T], BF16, tag="slt")   # [e, d, t]
            nc.vector.tensor_tensor(
                slt[:],
                loc[:].to_broadcast([P, T, P]).rearrange("p t d -> p d t"),
                cn["iota_dt"][:], op=OP.is_equal,
            )
            sga = slp.tile([P, T * P], BF16, tag="sga")  # [d, e] >= start
            nc.vector.tensor_scalar(
                sga[:], cn["iota_e"][:], se[:, 0:1], None, op0=OP.is_ge,
            )
            sgb = slp.tile([P, T * P], BF16, tag="sgb")  # [d, e] >= end
            nc.vector.tensor_scalar(
                sgb[:], cn["iota_e"][:], se[:, 1:2], None, op0=OP.is_ge,
            )
            s["slt"], s["sga"], s["sgb"] = slt, sga, sgb
            # alpha_dst for the block's 128 dsts: 256B-elem gather of the
            # tail half of the fat rows ([s | d | pad])
            adb = ip.tile([P, 1, P], BF16, tag="adb")
            nc.gpsimd.dma_gather(
                out_ap=adb[:], in_ap=tabf[:, HID:FAT], idxs_ap=ibt[:],
                num_idxs=P, num_idxs_reg=P, elem_size=P, elem_step=FAT,
                queue_num=qn[0] % 4,
            )
            qn[0] += 1
            adn = ip.tile([P, H], BF16, tag="adn")
            nc.vector.tensor_scalar(adn[:], adb[:, 0, H : 2 * H], -1.0, None,
                                    op0=OP.mult)
            s["adb"], s["adn"] = adb, adn
            # fat-row gather by src id
            gA = gp.tile([P, T, FAT], BF16, tag="gA")
            for c0 in range(0, T, CH):
                cw = min(CH, T - c0)
                nc.gpsimd.dma_gather(
                    out_ap=gA[:, c0 : c0 + cw, :], in_ap=tabf[:, :],
                    idxs_ap=ist[:, c0 * 8 : (c0 + cw) * 8],
                    num_idxs=P * cw, num_idxs_reg=P * cw, elem_size=FAT,
                    queue_num=qn[0] % 4,
                )
                qn[0] += 1
            s["gA"] = gA

        def front(b):
            s = st[b]
            gA, slt, sga, sgb = s["gA"], s["slt"], s["sga"], s["sgb"]
            adb, adn = s["adb"], s["adn"]
            # expand alpha_dst to edges: dx = (sga - sgb) @ adb on the PE
            dx = dpp.tile([P, T * H], F32, tag="dx")
            for t in range(T):
                nc.tensor.matmul(
                    dx[:, t * H : (t + 1) * H], sga[:, t * P : (t + 1) * P],
                    adb[:, 0, H : 2 * H], start=True, stop=False,
                )
                nc.tensor.matmul(
                    dx[:, t * H : (t + 1) * H], sgb[:, t * P : (t + 1) * P],
                    adn[:], start=False, stop=True,
                )
            ts = wp.tile([P, T, H], F32, tag="ts")
            nc.vector.tensor_tensor(
                ts[:], gA[:, :, HID : HID + H],
                dx[:].rearrange("p (t h) -> p t h", t=T), op=OP.add,
            )
            # ex = exp(leaky_relu(ts)): two chained ACT ops
            lr = wp.tile([P, T, H], F32, tag="lr")
            nc.scalar.activation(lr[:], ts[:], AF.Prelu, alpha=0.2)
            rhs = gp.tile([P, T, NW2], BF16, tag="rhs")
            exs = rhs[:, :, HID : HID + H]
            nc.scalar.activation(exs, lr[:], AF.Exp)
            nc.vector.tensor_tensor(
                rhs[:, :, 0:HID].rearrange("p t (h c) -> p t h c", h=H),
                gA[:, :, 0:HID].rearrange("p t (h c) -> p t h c", h=H),
                exs.rearrange("p t (h o) -> p t h o", h=H).to_broadcast(
                    [P, T, H, C]
                ),
                op=OP.mult,
            )
            s["rhs"] = rhs

        def back(b):
            s = st.pop(b)
            slt, rhs = s["slt"], s["rhs"]
            ps = pp.tile([P, NW2], F32, tag="eps")
            for t in range(T):
                nc.tensor.matmul(
                    ps[:], slt[:, :, t], rhs[:, t, :],
                    start=(t == 0), stop=(t == T - 1),
                )
            den = op_.tile([P, H], F32, tag="den")
            nc.vector.tensor_scalar_add(den[:], ps[:, HID : HID + H], 1e-16)
            rec = op_.tile([P, H], F32, tag="rec")
            nc.vector.reciprocal(rec[:], den[:])
            ot = op_.tile([P, HID], F32, tag="ot")
            nc.vector.tensor_tensor(
                ot[:].rearrange("p (h c) -> p h c", h=H),
                ps[:, 0:HID].rearrange("p (h c) -> p h c", h=H),
                rec[:].to_broadcast([P, H, C]),
                op=OP.mult,
            )
            nc.vector.tensor_tensor(ot[:], ot[:], bias_t[:], op=OP.add)
            if layer == 1:
                # ELU(x) = relu(x) + exp(min(x,0)) - 1 -> bf16 z
                r_ = op_.tile([P, HID], F32, tag="relu")
                nc.scalar.activation(r_[:], ot[:], AF.Relu)
                m_ = op_.tile([P, HID], F32, tag="mneg")
                nc.vector.tensor_tensor(m_[:], ot[:], r_[:], op=OP.subtract)
                nc.scalar.activation(m_[:], m_[:], AF.Exp)
                nc.vector.tensor_scalar_add(m_[:], m_[:], -1.0)
                zt = op_.tile([P, HID], BF16, tag="zt")
                nc.vector.tensor_tensor(zt[:], r_[:], m_[:], op=OP.add)
                # build this block's layer-2 table rows locally:
                # transpose z on the PE, then fold-in W2_ext
                psT = l2pp.tile([P, 2 * P], BF16, tag="psT")
                nc.tensor.transpose(psT[:, 0:P], zt[:, 0:P], cn["idn"][:])
                nc.tensor.transpose(psT[:, P : 2 * P], zt[:, P : 2 * P],
                                    cn["idn"][:])
                zT = l2sp.tile([P, 2 * P], BF16, tag="zT")
                nc.scalar.activation(zT[:], psT[:], AF.Copy)
                ps2 = l2pp.tile([P, cfg.NW], F32, tag="ps2")
                nc.tensor.matmul(ps2[:], zT[:, 0:P], cn["w2a"][:],
                                 start=True, stop=False)
                nc.tensor.matmul(ps2[:], zT[:, P : 2 * P], cn["w2b"][:],
                                 start=False, stop=True)
                f2 = l2sp.tile([P, FAT], BF16, tag="f2")
                nc.scalar.activation(f2[:, 0 : cfg.NW], ps2[:], AF.Copy)
                nc.scalar.dma_start(fat2own[b * P : (b + 1) * P, :], f2[:])
            else:
                nc.sync.dma_start(out_e[b * P : (b + 1) * P, :], ot[:])

        def maybe_ag(b):
            if layer != 1:
                return
            gs = BPC // NSPLIT
            if (b + 1) % gs == 0:
                f = b // gs
                HBR = cfg.HB
                nc.gpsimd.collective_compute(
                    "AllGather", OP.bypass,
                    replica_groups=[list(range(cfg.NC))],
                    ins=[fat2own[f * HBR : (f + 1) * HBR, :]],
                    outs=[tabf2sh[f * cfg.NC * HBR : (f + 1) * cfg.NC * HBR, :]],
                )

        prep(0)
        for i in range(BPC):
            if i + 1 < BPC:
                prep(i + 1)
            front(i)
            if i >= 1:
                back(i - 1)
                maybe_ag(i - 1)
        back(BPC - 1)
        maybe_ag(BPC - 1)


def build_program(cfg, T):
    nc = bacc.Bacc(num_swdge_queues=4)
    HID, NW, NP, NT, BPC = cfg.HID, cfg.NW, cfg.NP, cfg.NT, cfg.BPC

    xT_e = nc.declare_dram_parameter("xT", [HID, NP], BF16, isOutput=False)
    w1_e = nc.declare_dram_parameter("w1e", [HID, NW], BF16, isOutput=False)
    w2_e = nc.declare_dram_parameter("w2e", [HID, NW], BF16, isOutput=False)
    b1_e = nc.declare_dram_parameter("b1b", [P, HID], F32, isOutput=False)
    b2_e = nc.declare_dram_parameter("b2b", [P, HID], F32, isOutput=False)
    isrc_e = nc.declare_dram_parameter("isrc", [BPC, P, 8 * T], I16, isOutput=False)
    isrc2_e = nc.declare_dram_parameter("isrc2", [BPC, P, 8 * T], I16, isOutput=False)
    iblk_e = nc.declare_dram_parameter("iblk", [BPC, P, 8], I16, isOutput=False)
    iblk2_e = nc.declare_dram_parameter("iblk2", [BPC, P, 8], I16, isOutput=False)
    loc_e = nc.declare_dram_parameter("loce", [BPC, P, T], I16, isOutput=False)
    se_e = nc.declare_dram_parameter("sede", [BPC, P, 2], F32, isOutput=False)
    out_e = nc.declare_dram_parameter("out", [BPC * P, HID], F32, isOutput=True)

    tabf1 = nc.dram_tensor("tabf1", [NP, cfg.FAT], BF16)
    fat2own = nc.dram_tensor("fat2own", [BPC * P, cfg.FAT], BF16)
    tabf2sh = nc.dram_tensor("tabf2sh", [NP, cfg.FAT], BF16, addr_space="Shared")

    with tile.TileContext(nc) as tc:
        with tc.tile_pool(name="const", bufs=1) as cp:
            cn = {}
            for nm, src in (("w1a", w1_e), ("w1b", w1_e), ("w2a", w2_e),
                            ("w2b", w2_e)):
                t = cp.tile([P, NW], BF16, tag=nm)
                lo = 0 if nm.endswith("a") else P
                nc.sync.dma_start(t[:], src[lo : lo + P, :])
                cn[nm] = t
            b1t = cp.tile([P, HID], F32)
            nc.sync.dma_start(b1t[:], b1_e[:, :])
            cn["b1t"] = b1t
            b2t = cp.tile([P, HID], F32)
            nc.sync.dma_start(b2t[:], b2_e[:, :])
            cn["b2t"] = b2t
            idn = cp.tile([P, P], BF16)
            make_identity(nc, idn[:])
            cn["idn"] = idn
            iota_dt = cp.tile([P, P, T], I16)
            nc.gpsimd.iota(iota_dt[:], pattern=[[1, P], [0, T]],
                           channel_multiplier=0)
            cn["iota_dt"] = iota_dt
            iota_e = cp.tile([P, T * P], I16)
            nc.gpsimd.iota(iota_e[:], pattern=[[1, T * P]],
                           channel_multiplier=0)
            cn["iota_e"] = iota_e

            # ---- layer-1 tables (full, redundant per core) ----
            PAN = 10
            with (
                tc.tile_pool(name="s1", bufs=2) as sp,
                tc.tile_pool(name="fp1", bufs=4) as fp,
                tc.tile_pool(name="ps1", bufs=4, space="PSUM") as pp,
            ):
                for pan in range(-(-NT // PAN)):
                    j0, j1 = pan * PAN, min(NT, (pan + 1) * PAN)
                    w = (j1 - j0) * P
                    xp0 = sp.tile([P, PAN * P], BF16, tag="xp0")
                    nc.sync.dma_start(xp0[:, :w], xT_e[0:P, j0 * P : j1 * P])
                    xp1 = sp.tile([P, PAN * P], BF16, tag="xp1")
                    nc.sync.dma_start(xp1[:, :w], xT_e[P : 2 * P, j0 * P : j1 * P])
                    for j in range(j0, j1):
                        o = (j - j0) * P
                        ps = pp.tile([P, NW], F32, tag="tps")
                        nc.tensor.matmul(ps[:], xp0[:, o : o + P], cn["w1a"][:],
                                         start=True, stop=False)
                        nc.tensor.matmul(ps[:], xp1[:, o : o + P], cn["w1b"][:],
                                         start=False, stop=True)
                        fat = fp.tile([P, cfg.FAT], BF16, tag="fat")
                        nc.scalar.activation(fat[:, 0:NW], ps[:], AF.Copy)
                        nc.scalar.dma_start(tabf1[j * P : (j + 1) * P, :], fat[:])

            _edge_phase(nc, tc, cfg, T, cn, 1, tabf1, isrc_e, iblk_e, loc_e,
                        se_e, fat2own=fat2own, tabf2sh=tabf2sh)
            _edge_phase(nc, tc, cfg, T, cn, 2, tabf2sh, isrc2_e, iblk2_e,
                        loc_e, se_e, out_e=out_e)
    nc.finalize()
    return nc


# --------------------------------------------------------------------------
# Entry point
# --------------------------------------------------------------------------
def run_gat(inputs, cfg=None, trace=False):
    cfg = cfg or Cfg()
    in_maps, T = preprocess(cfg, **inputs)
    nc = build_program(cfg, T)
    res = run_bass_kernel_spmd(nc, in_maps, list(range(cfg.NC)), trace=trace)
    out = np.concatenate([res.results[c]["out"] for c in range(cfg.NC)], axis=0)
    return out[: cfg.N], res


def kernel(**inputs) -> np.ndarray:
    out, _ = run_gat(inputs)
    return np.ascontiguousarray(out, dtype=np.float32)


# revision 9
# speedup vs baseline: 1.4618x; 1.0034x over previous
"""2-layer 8-head GAT forward, distributed over 8 Trainium2 NeuronCores.

Strategy (graph data parallelism, per sharding hint):
  - Edges sorted by destination; dst nodes blocked by 128; 80 blocks sharded
    10-per-core. All index preprocessing is host-side (integers only).
  - Per layer each core holds ONE DRAM fat table (bf16):
      row n = [ h(n) (256) | alpha_src(n) (8) | alpha_dst(n) (8) | pad ] (768 B)
    built as h = x @ W_ext with attention vectors folded in
    (W_ext = [W | W@As | W@Ad]).
  - Edge phase per dst block (software-pipelined prep/front/back stages):
      * batched dma_gather of fat rows by src id (4 SWDGE queues)
      * alpha_dst rows for the block's own 128 dsts: one 256B-elem gather
      * selection matrices generated ON-CHIP from tiny index vectors, in
        DVE fast-mode friendly layouts (packed 16-bit last dims):
          selT[e,d,t] = (loc[e,t] == d)         one 2x DVE is_equal per block
          sga[d,e] = (e >= start[d])            4x DVE tensor_scalar
          sgb[d,e] = (e >= end[d])              4x DVE tensor_scalar
        (sld = sga - sgb; the subtraction is folded into the PE by
        accumulating sga@adb + sgb@(-adb))
      * ts = s + dx; ex = exp(lrelu(ts)) via two chained ACT ops;
        rhs = [ex*h | ex] (bf16); PSUM accumulation via selT matmuls (N=264).
      * epilogue divides by summed ex, adds bias (+ ELU after layer 1).
  - Layer-2 tables are built LOCALLY per core from its own z blocks
    (PE transpose of z + table matmul, no DRAM round-trip for z), then
    exchanged with five pipelined AllGathers (2 blocks each) overlapping
    the layer-1 edge phase tail. No per-block collectives.
Output: each core writes its 1280 dst rows f32; host concatenates and trims.
"""

import os
import sys

for _p in ("/opt/trn_rl_repo", "/root/.axon_site/_ro/trn_rl_repo"):
    if os.path.isdir(_p) and _p not in sys.path:
        sys.path.append(_p)

import numpy as np

from concourse import bacc, mybir
import concourse.tile as tile
from concourse.masks import make_identity
from concourse.bass_utils import run_bass_kernel_spmd

F32 = mybir.dt.float32
BF16 = mybir.dt.bfloat16
I16 = mybir.dt.int16
AF = mybir.ActivationFunctionType
OP = mybir.AluOpType
P = 128
AG_BOUNDS = [0, 4, 7, 9, 10]  # block boundaries of the pipelined AllGathers


class Cfg:
    def __init__(self, n_nodes=10000, n_edges=320000, hid=256, heads=8, n_cores=8):
        self.N = n_nodes
        self.E = n_edges
        self.HID = hid
        self.H = heads
        self.C = hid // heads
        self.NC = n_cores
        self.NP = -(-n_nodes // (P * n_cores)) * (P * n_cores)
        self.NT = self.NP // P
        self.BPC = self.NT // n_cores
        self.NW = hid + 3 * heads        # table matmul width (h | s | d | -d)
        self.NW2 = hid + heads           # edge matmul rhs width (ex*h | ex)
        self.FAT = hid + 128             # fat row elems (bf16): h | s | d | pad
        assert AG_BOUNDS[-1] == self.BPC
        assert self.FAT * 2 % 256 == 0


# --------------------------------------------------------------------------
# Host preprocessing (indices / weight folding)
# --------------------------------------------------------------------------
def _wrap16(idx):
    w = idx.reshape(-1, 16).T.astype(np.int16)
    return np.tile(w, (8, 1))


def _a_expand(a, cfg):
    A = np.zeros((cfg.H, cfg.C, cfg.H), np.float32)
    for h in range(cfg.H):
        A[h, :, h] = a[h]
    return A.reshape(cfg.HID, cfg.H)


def _remap2(n, cfg):
    """Row index of node n in the slice-wise AllGather'd layer-2 table."""
    r, l = np.divmod(n, cfg.BPC * P)
    bd = np.array([x * P for x in AG_BOUNDS])
    part = np.searchsorted(bd, l, side="right") - 1
    lo, hi = bd[part], bd[np.minimum(part + 1, len(bd) - 1)]
    return cfg.NC * lo + r * (hi - lo) + (l - lo)


def preprocess(cfg, x, edges_idx, W1, a_src1, a_dst1, b1, W2, a_src2, a_dst2, b2):
    import ml_dtypes

    bfd = ml_dtypes.bfloat16

    src = np.asarray(edges_idx[0], np.int64)
    dst = np.asarray(edges_idx[1], np.int64)
    order = np.argsort(dst, kind="stable")
    src_s, dst_s = src[order], dst[order]
    blk = dst_s // P
    counts = np.bincount(blk, minlength=cfg.NT)
    T = max(1, int(-(-counts.max() // P)))
    starts = np.concatenate([[0], np.cumsum(counts)])
    EPB = T * P

    isrc = np.zeros((cfg.NC, cfg.BPC, P, 8 * T), np.int16)
    isrc2 = np.zeros((cfg.NC, cfg.BPC, P, 8 * T), np.int16)
    iblk = np.zeros((cfg.NC, cfg.BPC, P, 8), np.int16)
    iblk2 = np.zeros((cfg.NC, cfg.BPC, P, 8), np.int16)
    loce = np.zeros((cfg.NC, cfg.BPC, P, T), np.int16)
    sede = np.zeros((cfg.NC, cfg.BPC, P, 2), np.float32)
    for gb in range(cfg.NT):
        c, b = gb // cfg.BPC, gb % cfg.BPC
        s0, s1 = starts[gb], starts[gb + 1]
        n = s1 - s0
        a_src = np.zeros(EPB, np.int64)
        a_loc = np.full(EPB, -1, np.int64)
        a_src[:n] = src_s[s0:s1]
        a_loc[:n] = dst_s[s0:s1] - gb * P
        isrc[c, b] = _wrap16(a_src)
        isrc2[c, b] = _wrap16(_remap2(a_src, cfg))
        own = np.arange(gb * P, (gb + 1) * P, dtype=np.int64)
        iblk[c, b] = _wrap16(own)
        iblk2[c, b] = _wrap16(_remap2(own, cfg))
        loce[c, b] = a_loc.reshape(T, P).T            # [e, t]
        # per-dst contiguous run bounds within the block's sorted edges
        cnt_d = np.bincount(a_loc[:n], minlength=P)
        end_d = np.cumsum(cnt_d)
        sede[c, b, :, 0] = (end_d - cnt_d).astype(np.float32)
        sede[c, b, :, 1] = end_d.astype(np.float32)

    Wd1 = W1 @ _a_expand(a_dst1, cfg)
    W1e = np.concatenate(
        [W1, W1 @ _a_expand(a_src1, cfg), Wd1, -Wd1], axis=1
    ).astype(np.float32)
    Wd2 = W2 @ _a_expand(a_dst2, cfg)
    W2e = np.concatenate(
        [W2, W2 @ _a_expand(a_src2, cfg), Wd2, -Wd2], axis=1
    ).astype(np.float32)

    xT = np.zeros((cfg.HID, cfg.NP), np.float32)
    xT[:, : cfg.N] = np.asarray(x, np.float32).T
    b1b = np.broadcast_to(np.asarray(b1, np.float32), (P, cfg.HID)).copy()
    b2b = np.broadcast_to(np.asarray(b2, np.float32), (P, cfg.HID)).copy()

    shared = {
        "xT": xT.astype(bfd), "w1e": W1e.astype(bfd), "w2e": W2e.astype(bfd),
        "b1b": b1b, "b2b": b2b,
    }
    in_maps = [
        dict(shared, isrc=isrc[c], isrc2=isrc2[c], iblk=iblk[c], iblk2=iblk2[c],
             loce=loce[c], sede=sede[c])
        for c in range(cfg.NC)
    ]
    return in_maps, T


# --------------------------------------------------------------------------
# Device program
# --------------------------------------------------------------------------
def _edge_phase(nc, tc, cfg, T, cn, layer, tabf, isrc_e, iblk_e, loc_e, se_e,
                fat2own=None, tabf2sh=None, out_e=None):
    """Edge phase for one layer, software-pipelined over this core's blocks.

    layer==1: epilogue applies ELU, builds the layer-2 table rows for the
    block locally (PE transpose + matmul) and stages them for AllGather.
    layer==2: epilogue writes the final f32 output rows.
    """
    HID, H, C, NW2, FAT, BPC = cfg.HID, cfg.H, cfg.C, cfg.NW2, cfg.FAT, cfg.BPC
    CH = 8  # 1024-idx gather chunks (HW limit)
    qn = [0]
    bias_t = cn["b1t"] if layer == 1 else cn["b2t"]
    st = {}  # per-block live tiles

    with (
        tc.tile_pool(name=f"ge{layer}", bufs=2) as gp,
        tc.tile_pool(name=f"ix{layer}", bufs=3) as ip,
        tc.tile_pool(name=f"sl{layer}", bufs=2) as slp,
        tc.tile_pool(name=f"wk{layer}", bufs=2) as wp,
        tc.tile_pool(name=f"eo{layer}", bufs=2) as op_,
        tc.tile_pool(name=f"eps{layer}", bufs=2, space="PSUM") as pp,
        tc.tile_pool(name=f"dps{layer}", bufs=2, space="PSUM") as dpp,
        tc.tile_pool(name=f"l2ps{layer}", bufs=1, space="PSUM") as l2pp,
        tc.tile_pool(name=f"l2sb{layer}", bufs=3) as l2sp,
    ):
        def loads(b):
            s = st[b] = {}
            ist = ip.tile([P, 8 * T], I16, tag="isrc")
            nc.sync.dma_start(ist[:], isrc_e[b])
            loc = ip.tile([P, T], I16, tag="loc")
            nc.sync.dma_start(loc[:], loc_e[b])
            se = ip.tile([P, 2], F32, tag="se")
            nc.sync.dma_start(se[:], se_e[b])
            ibt = ip.tile([P, 8], I16, tag="iblk")
            nc.sync.dma_start(ibt[:], iblk_e[b])
            s["ist"], s["loc"], s["se"], s["ibt"] = ist, loc, se, ibt

        def prep(b):
            s = st[b]
            ist, loc, se, ibt = s["ist"], s["loc"], s["se"], s["ibt"]
            # on-chip selection masks (all operands packed 16-bit last dim)
            slt = slp.tile([P, P, T], BF16, tag="slt")   # [e, d, t]
            nc.vector.tensor_tensor(
                slt[:],
                loc[:].to_broadcast([P, T, P]).rearrange("p t d -> p d t"),
                cn["iota_dt"][:], op=OP.is_equal,
            )
            sga = slp.tile([P, T * P], BF16, tag="sga")  # [d, e] >= start
            nc.vector.tensor_scalar(
                sga[:], cn["iota_e"][:], se[:, 0:1], None, op0=OP.is_ge,
            )
            sgb = slp.tile([P, T * P], BF16, tag="sgb")  # [d, e] >= end
            nc.vector.tensor_scalar(
                sgb[:], cn["iota_e"][:], se[:, 1:2], None, op0=OP.is_ge,
            )
            s["slt"], s["sga"], s["sgb"] = slt, sga, sgb
            # alpha_dst for the block's 128 dsts: 256B-elem gather of the
            # tail half of the fat rows ([s | d | pad])
            adb = ip.tile([P, 1, P], BF16, tag="adb")
            nc.gpsimd.dma_gather(
                out_ap=adb[:], in_ap=tabf[:, HID:FAT], idxs_ap=ibt[:],
                num_idxs=P, num_idxs_reg=P, elem_size=P, elem_step=FAT,
                queue_num=qn[0] % 4,
            )
            qn[0] += 1
            s["adb"] = adb
            # fat-row gather by src id
            gA = gp.tile([P, T, FAT], BF16, tag="gA")
            for c0 in range(0, T, CH):
                cw = min(CH, T - c0)
                nc.gpsimd.dma_gather(
                    out_ap=gA[:, c0 : c0 + cw, :], in_ap=tabf[:, :],
                    idxs_ap=ist[:, c0 * 8 : (c0 + cw) * 8],
                    num_idxs=P * cw, num_idxs_reg=P * cw, elem_size=FAT,
                    queue_num=qn[0] % 4,
                )
                qn[0] += 1
            s["gA"] = gA

        def front(b):
            s = st[b]
            gA, slt, sga, sgb = s["gA"], s["slt"], s["sga"], s["sgb"]
            adb = s["adb"]
            # expand alpha_dst to edges: dx = (sga - sgb) @ adb on the PE
            dx = dpp.tile([P, T * H], F32, tag="dx")
            for t in range(T):
                nc.tensor.matmul(
                    dx[:, t * H : (t + 1) * H], sga[:, t * P : (t + 1) * P],
                    adb[:, 0, H : 2 * H], start=True, stop=False,
                )
                nc.tensor.matmul(
                    dx[:, t * H : (t + 1) * H], sgb[:, t * P : (t + 1) * P],
                    adb[:, 0, 2 * H : 3 * H], start=False, stop=True,
                )
            ts = wp.tile([P, T, H], F32, tag="ts")
            nc.vector.tensor_tensor(
                ts[:], gA[:, :, HID : HID + H],
                dx[:].rearrange("p (t h) -> p t h", t=T), op=OP.add,
            )
            # ex = exp(leaky_relu(ts)): two chained ACT ops
            lr = wp.tile([P, T, H], F32, tag="lr")
            nc.scalar.activation(lr[:], ts[:], AF.Prelu, alpha=0.2)
            rhs = gp.tile([P, T, NW2], BF16, tag="rhs")
            exs = rhs[:, :, HID : HID + H]
            nc.scalar.activation(exs, lr[:], AF.Exp)
            nc.vector.tensor_tensor(
                rhs[:, :, 0:HID].rearrange("p t (h c) -> p t h c", h=H),
                gA[:, :, 0:HID].rearrange("p t (h c) -> p t h c", h=H),
                exs.rearrange("p t (h o) -> p t h o", h=H).to_broadcast(
                    [P, T, H, C]
                ),
                op=OP.mult,
            )
            s["rhs"] = rhs

        def back(b):
            s = st.pop(b)
            slt, rhs = s["slt"], s["rhs"]
            ps = pp.tile([P, NW2], F32, tag="eps")
            for t in range(T):
                nc.tensor.matmul(
                    ps[:], slt[:, :, t], rhs[:, t, :],
                    start=(t == 0), stop=(t == T - 1),
                )
            den = op_.tile([P, H], F32, tag="den")
            nc.vector.tensor_scalar_add(den[:], ps[:, HID : HID + H], 1e-16)
            rec = op_.tile([P, H], F32, tag="rec")
            nc.vector.reciprocal(rec[:], den[:])
            ot = op_.tile([P, HID], F32, tag="ot")
            nc.vector.tensor_tensor(
                ot[:].rearrange("p (h c) -> p h c", h=H),
                ps[:, 0:HID].rearrange("p (h c) -> p h c", h=H),
                rec[:].to_broadcast([P, H, C]),
                op=OP.mult,
            )
            nc.vector.tensor_tensor(ot[:], ot[:], bias_t[:], op=OP.add)
            if layer == 1:
                # ELU(x) = relu(x) + exp(min(x,0)) - 1 -> bf16 z
                r_ = op_.tile([P, HID], F32, tag="relu")
                nc.scalar.activation(r_[:], ot[:], AF.Relu)
                m_ = op_.tile([P, HID], F32, tag="mneg")
                nc.vector.tensor_tensor(m_[:], ot[:], r_[:], op=OP.subtract)
                nc.scalar.activation(m_[:], m_[:], AF.Exp)
                nc.vector.tensor_scalar_add(m_[:], m_[:], -1.0)
                zt = op_.tile([P, HID], BF16, tag="zt")
                nc.vector.tensor_tensor(zt[:], r_[:], m_[:], op=OP.add)
                # build this block's layer-2 table rows locally:
                # transpose z on the PE, then fold-in W2_ext
                psT = l2pp.tile([P, 2 * P], BF16, tag="psT")
                nc.tensor.transpose(psT[:, 0:P], zt[:, 0:P], cn["idn"][:])
                nc.tensor.transpose(psT[:, P : 2 * P], zt[:, P : 2 * P],
                                    cn["idn"][:])
                zT = l2sp.tile([P, 2 * P], BF16, tag="zT")
                nc.scalar.activation(zT[:], psT[:], AF.Copy)
                ps2 = l2pp.tile([P, cfg.NW], F32, tag="ps2")
                nc.tensor.matmul(ps2[:], zT[:, 0:P], cn["w2a"][:],
                                 start=True, stop=False)
                nc.tensor.matmul(ps2[:], zT[:, P : 2 * P], cn["w2b"][:],
                                 start=False, stop=True)
                f2 = l2sp.tile([P, FAT], BF16, tag="f2")
                nc.scalar.activation(f2[:, 0 : cfg.NW], ps2[:], AF.Copy)
                nc.scalar.dma_start(fat2own[b * P : (b + 1) * P, :], f2[:])
            else:
                nc.sync.dma_start(out_e[b * P : (b + 1) * P, :], ot[:])

        def maybe_ag(b):
            if layer != 1:
                return
            bd = [x * P for x in AG_BOUNDS]
            for k in range(len(bd) - 1):
                if b + 1 == bd[k + 1] // P:
                    nc.gpsimd.collective_compute(
                        "AllGather", OP.bypass,
                        replica_groups=[list(range(cfg.NC))],
                        ins=[fat2own[bd[k] : bd[k + 1], :]],
                        outs=[tabf2sh[cfg.NC * bd[k] : cfg.NC * bd[k + 1], :]],
                    )

        loads(0)
        loads(1)
        prep(0)
        for i in range(BPC):
            if i + 2 < BPC:
                loads(i + 2)
            if i + 1 < BPC:
                prep(i + 1)
            front(i)
            if i >= 1:
                back(i - 1)
                maybe_ag(i - 1)
        back(BPC - 1)
        maybe_ag(BPC - 1)


def build_program(cfg, T):
    nc = bacc.Bacc(num_swdge_queues=4)
    HID, NW, NP, NT, BPC = cfg.HID, cfg.NW, cfg.NP, cfg.NT, cfg.BPC

    xT_e = nc.declare_dram_parameter("xT", [HID, NP], BF16, isOutput=False)
    w1_e = nc.declare_dram_parameter("w1e", [HID, NW], BF16, isOutput=False)
    w2_e = nc.declare_dram_parameter("w2e", [HID, NW], BF16, isOutput=False)
    b1_e = nc.declare_dram_parameter("b1b", [P, HID], F32, isOutput=False)
    b2_e = nc.declare_dram_parameter("b2b", [P, HID], F32, isOutput=False)
    isrc_e = nc.declare_dram_parameter("isrc", [BPC, P, 8 * T], I16, isOutput=False)
    isrc2_e = nc.declare_dram_parameter("isrc2", [BPC, P, 8 * T], I16, isOutput=False)
    iblk_e = nc.declare_dram_parameter("iblk", [BPC, P, 8], I16, isOutput=False)
    iblk2_e = nc.declare_dram_parameter("iblk2", [BPC, P, 8], I16, isOutput=False)
    loc_e = nc.declare_dram_parameter("loce", [BPC, P, T], I16, isOutput=False)
    se_e = nc.declare_dram_parameter("sede", [BPC, P, 2], F32, isOutput=False)
    out_e = nc.declare_dram_parameter("out", [BPC * P, HID], F32, isOutput=True)

    tabf1 = nc.dram_tensor("tabf1", [NP, cfg.FAT], BF16)
    fat2own = nc.dram_tensor("fat2own", [BPC * P, cfg.FAT], BF16)
    tabf2sh = nc.dram_tensor("tabf2sh", [NP, cfg.FAT], BF16, addr_space="Shared")

    with tile.TileContext(nc) as tc:
        with tc.tile_pool(name="const", bufs=1) as cp:
            cn = {}
            for nm, src in (("w1a", w1_e), ("w1b", w1_e), ("w2a", w2_e),
                            ("w2b", w2_e)):
                t = cp.tile([P, NW], BF16, tag=nm)
                lo = 0 if nm.endswith("a") else P
                nc.sync.dma_start(t[:], src[lo : lo + P, :])
                cn[nm] = t
            b1t = cp.tile([P, HID], F32)
            nc.sync.dma_start(b1t[:], b1_e[:, :])
            cn["b1t"] = b1t
            b2t = cp.tile([P, HID], F32)
            nc.sync.dma_start(b2t[:], b2_e[:, :])
            cn["b2t"] = b2t
            idn = cp.tile([P, P], BF16)
            make_identity(nc, idn[:])
            cn["idn"] = idn
            iota_dt = cp.tile([P, P, T], I16)
            nc.gpsimd.iota(iota_dt[:], pattern=[[1, P], [0, T]],
                           channel_multiplier=0)
            cn["iota_dt"] = iota_dt
            iota_e = cp.tile([P, T * P], I16)
            nc.gpsimd.iota(iota_e[:], pattern=[[1, T * P]],
                           channel_multiplier=0)
            cn["iota_e"] = iota_e

            # ---- layer-1 tables (full, redundant per core) ----
            PAN = 10
            with (
                tc.tile_pool(name="s1", bufs=2) as sp,
                tc.tile_pool(name="fp1", bufs=4) as fp,
                tc.tile_pool(name="ps1", bufs=4, space="PSUM") as pp,
            ):
                for pan in range(-(-NT // PAN)):
                    j0, j1 = pan * PAN, min(NT, (pan + 1) * PAN)
                    w = (j1 - j0) * P
                    xp0 = sp.tile([P, PAN * P], BF16, tag="xp0")
                    nc.sync.dma_start(xp0[:, :w], xT_e[0:P, j0 * P : j1 * P])
                    xp1 = sp.tile([P, PAN * P], BF16, tag="xp1")
                    nc.sync.dma_start(xp1[:, :w], xT_e[P : 2 * P, j0 * P : j1 * P])
                    for j in range(j0, j1):
                        o = (j - j0) * P
                        ps = pp.tile([P, NW], F32, tag="tps")
                        nc.tensor.matmul(ps[:], xp0[:, o : o + P], cn["w1a"][:],
                                         start=True, stop=False)
                        nc.tensor.matmul(ps[:], xp1[:, o : o + P], cn["w1b"][:],
                                         start=False, stop=True)
                        fat = fp.tile([P, cfg.FAT], BF16, tag="fat")
                        nc.scalar.activation(fat[:, 0:NW], ps[:], AF.Copy)
                        nc.scalar.dma_start(tabf1[j * P : (j + 1) * P, :], fat[:])

            _edge_phase(nc, tc, cfg, T, cn, 1, tabf1, isrc_e, iblk_e, loc_e,
                        se_e, fat2own=fat2own, tabf2sh=tabf2sh)
            _edge_phase(nc, tc, cfg, T, cn, 2, tabf2sh, isrc2_e, iblk2_e,
                        loc_e, se_e, out_e=out_e)
    nc.finalize()
    return nc


# --------------------------------------------------------------------------
# Entry point
# --------------------------------------------------------------------------
def run_gat(inputs, cfg=None, trace=False):
    cfg = cfg or Cfg()
    in_maps, T = preprocess(cfg, **inputs)
    nc = build_program(cfg, T)
    res = run_bass_kernel_spmd(nc, in_maps, list(range(cfg.NC)), trace=trace)
    out = np.concatenate([res.results[c]["out"] for c in range(cfg.NC)], axis=0)
    return out[: cfg.N], res


def kernel(**inputs) -> np.ndarray:
    out, _ = run_gat(inputs)
    return np.ascontiguousarray(out, dtype=np.float32)


# revision 11
# speedup vs baseline: 1.5057x; 1.0301x over previous
"""2-layer 8-head GAT forward, distributed over 8 Trainium2 NeuronCores.

Strategy (graph data parallelism, per sharding hint):
  - Edges sorted by destination; dst nodes blocked by 128; 80 blocks sharded
    10-per-core. All index preprocessing is host-side (integers only).
  - Per layer each core holds ONE DRAM fat table (bf16):
      row n = [ h(n) (256) | alpha_src(n) (8) | alpha_dst(n) (8) | pad ] (768 B)
    built as h = x @ W_ext with attention vectors folded in
    (W_ext = [W | W@As | W@Ad]).
  - Edge phase per dst block (software-pipelined prep/front/back stages):
      * batched dma_gather of fat rows by src id (4 SWDGE queues)
      * alpha_dst rows for the block's own 128 dsts: one 256B-elem gather
      * selection matrices generated ON-CHIP from tiny index vectors, in
        DVE fast-mode friendly layouts (packed 16-bit last dims):
          selT[e,d,t] = (loc[e,t] == d)         one 2x DVE is_equal per block
          sga[d,e] = (e >= start[d])            4x DVE tensor_scalar
          sgb[d,e] = (e >= end[d])              4x DVE tensor_scalar
        (sld = sga - sgb; the subtraction is folded into the PE by
        accumulating sga@adb + sgb@(-adb))
      * ts = s + dx; ex = exp(lrelu(ts)) via two chained ACT ops;
        rhs = [ex*h | ex] (bf16); PSUM accumulation via selT matmuls (N=264).
      * epilogue divides by summed ex, adds bias (+ ELU after layer 1).
  - Layer-2 tables are built LOCALLY per core from its own z blocks
    (PE transpose of z + table matmul, no DRAM round-trip for z), then
    exchanged with five pipelined AllGathers (2 blocks each) overlapping
    the layer-1 edge phase tail. No per-block collectives.
Output: each core writes its 1280 dst rows f32; host concatenates and trims.
"""

import os
import sys

for _p in ("/opt/trn_rl_repo", "/root/.axon_site/_ro/trn_rl_repo"):
    if os.path.isdir(_p) and _p not in sys.path:
        sys.path.append(_p)

import numpy as np

from concourse import bacc, mybir
import concourse.tile as tile
from concourse.masks import make_identity
from concourse.bass_utils import run_bass_kernel_spmd

F32 = mybir.dt.float32
BF16 = mybir.dt.bfloat16
I16 = mybir.dt.int16
AF = mybir.ActivationFunctionType
OP = mybir.AluOpType
P = 128
AG_BOUNDS = [0, 2, 4, 6, 8, 9, 10]  # block boundaries of the pipelined AllGathers


class Cfg:
    def __init__(self, n_nodes=10000, n_edges=320000, hid=256, heads=8, n_cores=8):
        self.N = n_nodes
        self.E = n_edges
        self.HID = hid
        self.H = heads
        self.C = hid // heads
        self.NC = n_cores
        self.NP = -(-n_nodes // (P * n_cores)) * (P * n_cores)
        self.NT = self.NP // P
        self.BPC = self.NT // n_cores
        self.NW = hid + 3 * heads        # table matmul width (h | s | d | -d)
        self.NW2 = hid + heads           # edge matmul rhs width (ex*h | ex)
        self.FAT = hid + 128             # fat row elems (bf16): h | s | d | pad
        assert AG_BOUNDS[-1] == self.BPC
        assert self.FAT * 2 % 256 == 0


# --------------------------------------------------------------------------
# Host preprocessing (indices / weight folding)
# --------------------------------------------------------------------------
def _wrap16(idx):
    w = idx.reshape(-1, 16).T.astype(np.int16)
    return np.tile(w, (8, 1))


def _a_expand(a, cfg):
    A = np.zeros((cfg.H, cfg.C, cfg.H), np.float32)
    for h in range(cfg.H):
        A[h, :, h] = a[h]
    return A.reshape(cfg.HID, cfg.H)


def _remap2(n, cfg):
    """Row index of node n in the slice-wise AllGather'd layer-2 table."""
    r, l = np.divmod(n, cfg.BPC * P)
    bd = np.array([x * P for x in AG_BOUNDS])
    part = np.searchsorted(bd, l, side="right") - 1
    lo, hi = bd[part], bd[np.minimum(part + 1, len(bd) - 1)]
    return cfg.NC * lo + r * (hi - lo) + (l - lo)


def preprocess(cfg, x, edges_idx, W1, a_src1, a_dst1, b1, W2, a_src2, a_dst2, b2):
    import ml_dtypes

    bfd = ml_dtypes.bfloat16

    src = np.asarray(edges_idx[0], np.int64)
    dst = np.asarray(edges_idx[1], np.int64)
    order = np.argsort(dst, kind="stable")
    src_s, dst_s = src[order], dst[order]
    blk = dst_s // P
    counts = np.bincount(blk, minlength=cfg.NT)
    T = max(1, int(-(-counts.max() // P)))
    starts = np.concatenate([[0], np.cumsum(counts)])
    EPB = T * P

    isrc = np.zeros((cfg.NC, cfg.BPC, P, 8 * T), np.int16)
    isrc2 = np.zeros((cfg.NC, cfg.BPC, P, 8 * T), np.int16)
    iblk = np.zeros((cfg.NC, cfg.BPC, P, 8), np.int16)
    iblk2 = np.zeros((cfg.NC, cfg.BPC, P, 8), np.int16)
    loce = np.zeros((cfg.NC, cfg.BPC, P, T), np.int16)
    sede = np.zeros((cfg.NC, cfg.BPC, P, 2), np.float32)
    for gb in range(cfg.NT):
        c, b = gb // cfg.BPC, gb % cfg.BPC
        s0, s1 = starts[gb], starts[gb + 1]
        n = s1 - s0
        a_src = np.zeros(EPB, np.int64)
        a_loc = np.full(EPB, -1, np.int64)
        a_src[:n] = src_s[s0:s1]
        a_loc[:n] = dst_s[s0:s1] - gb * P
        isrc[c, b] = _wrap16(a_src)
        isrc2[c, b] = _wrap16(_remap2(a_src, cfg))
        own = np.arange(gb * P, (gb + 1) * P, dtype=np.int64)
        iblk[c, b] = _wrap16(own)
        iblk2[c, b] = _wrap16(_remap2(own, cfg))
        loce[c, b] = a_loc.reshape(T, P).T            # [e, t]
        # per-dst contiguous run bounds within the block's sorted edges
        cnt_d = np.bincount(a_loc[:n], minlength=P)
        end_d = np.cumsum(cnt_d)
        sede[c, b, :, 0] = (end_d - cnt_d).astype(np.float32)
        sede[c, b, :, 1] = end_d.astype(np.float32)

    Wd1 = W1 @ _a_expand(a_dst1, cfg)
    W1e = np.concatenate(
        [W1, W1 @ _a_expand(a_src1, cfg), Wd1, -Wd1], axis=1
    ).astype(np.float32)
    Wd2 = W2 @ _a_expand(a_dst2, cfg)
    W2e = np.concatenate(
        [W2, W2 @ _a_expand(a_src2, cfg), Wd2, -Wd2], axis=1
    ).astype(np.float32)

    xT = np.zeros((cfg.HID, cfg.NP), np.float32)
    xT[:, : cfg.N] = np.asarray(x, np.float32).T
    b1b = np.broadcast_to(np.asarray(b1, np.float32), (P, cfg.HID)).copy()
    b2b = np.broadcast_to(np.asarray(b2, np.float32), (P, cfg.HID)).copy()

    shared = {
        "xT": xT.astype(bfd), "w1e": W1e.astype(bfd), "w2e": W2e.astype(bfd),
        "b1b": b1b, "b2b": b2b,
    }
    in_maps = [
        dict(shared, isrc=isrc[c], isrc2=isrc2[c], iblk=iblk[c], iblk2=iblk2[c],
             loce=loce[c], sede=sede[c])
        for c in range(cfg.NC)
    ]
    return in_maps, T


# --------------------------------------------------------------------------
# Device program
# --------------------------------------------------------------------------
def _edge_phase(nc, tc, cfg, T, cn, layer, tabf, isrc_e, iblk_e, loc_e, se_e,
                fat2own=None, tabf2sh=None, out_e=None):
    """Edge phase for one layer, software-pipelined over this core's blocks.

    layer==1: epilogue applies ELU, builds the layer-2 table rows for the
    block locally (PE transpose + matmul) and stages them for AllGather.
    layer==2: epilogue writes the final f32 output rows.
    """
    HID, H, C, NW2, FAT, BPC = cfg.HID, cfg.H, cfg.C, cfg.NW2, cfg.FAT, cfg.BPC
    CH = 8  # 1024-idx gather chunks (HW limit)
    qn = [0]
    bias_t = cn["b1t"] if layer == 1 else cn["b2t"]
    st = {}  # per-block live tiles

    with (
        tc.tile_pool(name=f"ge{layer}", bufs=3) as gp,
        tc.tile_pool(name=f"rh{layer}", bufs=2) as rp,
        tc.tile_pool(name=f"ix{layer}", bufs=3) as ip,
        tc.tile_pool(name=f"sl{layer}", bufs=2) as slp,
        tc.tile_pool(name=f"wk{layer}", bufs=2) as wp,
        tc.tile_pool(name=f"eo{layer}", bufs=2) as op_,
        tc.tile_pool(name=f"eps{layer}", bufs=2, space="PSUM") as pp,
        tc.tile_pool(name=f"dps{layer}", bufs=2, space="PSUM") as dpp,
        tc.tile_pool(name=f"l2ps{layer}", bufs=1, space="PSUM") as l2pp,
        tc.tile_pool(name=f"l2sb{layer}", bufs=3) as l2sp,
    ):
        def loads(b):
            s = st[b] = {}
            ist = ip.tile([P, 8 * T], I16, tag="isrc")
            nc.sync.dma_start(ist[:], isrc_e[b])
            loc = ip.tile([P, T], I16, tag="loc")
            nc.sync.dma_start(loc[:], loc_e[b])
            se = ip.tile([P, 2], F32, tag="se")
            nc.sync.dma_start(se[:], se_e[b])
            ibt = ip.tile([P, 8], I16, tag="iblk")
            nc.sync.dma_start(ibt[:], iblk_e[b])
            s["ist"], s["loc"], s["se"], s["ibt"] = ist, loc, se, ibt

        def prep(b):
            s = st[b]
            ist, loc, se, ibt = s["ist"], s["loc"], s["se"], s["ibt"]
            # on-chip selection masks (all operands packed 16-bit last dim)
            slt = slp.tile([P, P, T], BF16, tag="slt")   # [e, d, t]
            nc.vector.tensor_tensor(
                slt[:],
                loc[:].to_broadcast([P, T, P]).rearrange("p t d -> p d t"),
                cn["iota_dt"][:], op=OP.is_equal,
            )
            sga = slp.tile([P, T * P], BF16, tag="sga")  # [d, e] >= start
            nc.vector.tensor_scalar(
                sga[:], cn["iota_e"][:], se[:, 0:1], None, op0=OP.is_ge,
            )
            sgb = slp.tile([P, T * P], BF16, tag="sgb")  # [d, e] >= end
            nc.vector.tensor_scalar(
                sgb[:], cn["iota_e"][:], se[:, 1:2], None, op0=OP.is_ge,
            )
            s["slt"], s["sga"], s["sgb"] = slt, sga, sgb
            # alpha_dst for the block's 128 dsts: 256B-elem gather of the
            # tail half of the fat rows ([s | d | pad])
            adb = ip.tile([P, 1, P], BF16, tag="adb")
            nc.gpsimd.dma_gather(
                out_ap=adb[:], in_ap=tabf[:, HID:FAT], idxs_ap=ibt[:],
                num_idxs=P, num_idxs_reg=P, elem_size=P, elem_step=FAT,
                queue_num=qn[0] % 4,
            )
            qn[0] += 1
            s["adb"] = adb
            # fat-row gather by src id
            gA = gp.tile([P, T, FAT], BF16, tag="gA")
            for c0 in range(0, T, CH):
                cw = min(CH, T - c0)
                nc.gpsimd.dma_gather(
                    out_ap=gA[:, c0 : c0 + cw, :], in_ap=tabf[:, :],
                    idxs_ap=ist[:, c0 * 8 : (c0 + cw) * 8],
                    num_idxs=P * cw, num_idxs_reg=P * cw, elem_size=FAT,
                    queue_num=qn[0] % 4,
                )
                qn[0] += 1
            s["gA"] = gA

        def front(b):
            s = st[b]
            gA, slt, sga, sgb = s["gA"], s["slt"], s["sga"], s["sgb"]
            adb = s["adb"]
            # expand alpha_dst to edges: dx = (sga - sgb) @ adb on the PE
            dx = dpp.tile([P, T * H], F32, tag="dx")
            for t in range(T):
                nc.tensor.matmul(
                    dx[:, t * H : (t + 1) * H], sga[:, t * P : (t + 1) * P],
                    adb[:, 0, H : 2 * H], start=True, stop=False,
                )
                nc.tensor.matmul(
                    dx[:, t * H : (t + 1) * H], sgb[:, t * P : (t + 1) * P],
                    adb[:, 0, 2 * H : 3 * H], start=False, stop=True,
                )
            ts = wp.tile([P, T, H], F32, tag="ts")
            nc.vector.tensor_tensor(
                ts[:], gA[:, :, HID : HID + H],
                dx[:].rearrange("p (t h) -> p t h", t=T), op=OP.add,
            )
            # ex = exp(leaky_relu(ts)): two chained ACT ops
            lr = wp.tile([P, T, H], F32, tag="lr")
            nc.scalar.activation(lr[:], ts[:], AF.Prelu, alpha=0.2)
            rhs = rp.tile([P, T, NW2], BF16, tag="rhs")
            exs = rhs[:, :, HID : HID + H]
            nc.scalar.activation(exs, lr[:], AF.Exp)
            nc.vector.tensor_tensor(
                rhs[:, :, 0:HID].rearrange("p t (h c) -> p t h c", h=H),
                gA[:, :, 0:HID].rearrange("p t (h c) -> p t h c", h=H),
                exs.rearrange("p t (h o) -> p t h o", h=H).to_broadcast(
                    [P, T, H, C]
                ),
                op=OP.mult,
            )
            s["rhs"] = rhs

        def back(b):
            s = st.pop(b)
            slt, rhs = s["slt"], s["rhs"]
            ps = pp.tile([P, NW2], F32, tag="eps")
            for t in range(T):
                nc.tensor.matmul(
                    ps[:], slt[:, :, t], rhs[:, t, :],
                    start=(t == 0), stop=(t == T - 1),
                )
            den = op_.tile([P, H], F32, tag="den")
            nc.vector.tensor_scalar_add(den[:], ps[:, HID : HID + H], 1e-16)
            rec = op_.tile([P, H], F32, tag="rec")
            nc.vector.reciprocal(rec[:], den[:])
            ot = op_.tile([P, HID], F32, tag="ot")
            nc.vector.tensor_tensor(
                ot[:].rearrange("p (h c) -> p h c", h=H),
                ps[:, 0:HID].rearrange("p (h c) -> p h c", h=H),
                rec[:].to_broadcast([P, H, C]),
                op=OP.mult,
            )
            nc.vector.tensor_tensor(ot[:], ot[:], bias_t[:], op=OP.add)
            if layer == 1:
                # ELU(x) = relu(x) + exp(min(x,0)) - 1 -> bf16 z
                r_ = op_.tile([P, HID], F32, tag="relu")
                nc.scalar.activation(r_[:], ot[:], AF.Relu)
                m_ = op_.tile([P, HID], F32, tag="mneg")
                nc.vector.tensor_tensor(m_[:], ot[:], r_[:], op=OP.subtract)
                nc.scalar.activation(m_[:], m_[:], AF.Exp)
                nc.vector.tensor_scalar_add(m_[:], m_[:], -1.0)
                zt = op_.tile([P, HID], BF16, tag="zt")
                nc.vector.tensor_tensor(zt[:], r_[:], m_[:], op=OP.add)
                # build this block's layer-2 table rows locally:
                # transpose z on the PE, then fold-in W2_ext
                psT = l2pp.tile([P, 2 * P], BF16, tag="psT")
                nc.tensor.transpose(psT[:, 0:P], zt[:, 0:P], cn["idn"][:])
                nc.tensor.transpose(psT[:, P : 2 * P], zt[:, P : 2 * P],
                                    cn["idn"][:])
                zT = l2sp.tile([P, 2 * P], BF16, tag="zT")
                nc.scalar.activation(zT[:], psT[:], AF.Copy)
                ps2 = l2pp.tile([P, cfg.NW], F32, tag="ps2")
                nc.tensor.matmul(ps2[:], zT[:, 0:P], cn["w2a"][:],
                                 start=True, stop=False)
                nc.tensor.matmul(ps2[:], zT[:, P : 2 * P], cn["w2b"][:],
                                 start=False, stop=True)
                f2 = l2sp.tile([P, FAT], BF16, tag="f2")
                nc.scalar.activation(f2[:, 0 : cfg.NW], ps2[:], AF.Copy)
                nc.scalar.dma_start(fat2own[b * P : (b + 1) * P, :], f2[:])
            else:
                nc.sync.dma_start(out_e[b * P : (b + 1) * P, :], ot[:])

        def maybe_ag(b):
            if layer != 1:
                return
            bd = [x * P for x in AG_BOUNDS]
            for k in range(len(bd) - 1):
                if b + 1 == bd[k + 1] // P:
                    nc.gpsimd.collective_compute(
                        "AllGather", OP.bypass,
                        replica_groups=[list(range(cfg.NC))],
                        ins=[fat2own[bd[k] : bd[k + 1], :]],
                        outs=[tabf2sh[cfg.NC * bd[k] : cfg.NC * bd[k + 1], :]],
                    )

        loads(0)
        loads(1)
        prep(0)
        for i in range(BPC):
            if i + 2 < BPC:
                loads(i + 2)
            if i + 1 < BPC:
                prep(i + 1)
            front(i)
            if i >= 1:
                back(i - 1)
                maybe_ag(i - 1)
        back(BPC - 1)
        maybe_ag(BPC - 1)


def build_program(cfg, T):
    nc = bacc.Bacc(num_swdge_queues=4)
    HID, NW, NP, NT, BPC = cfg.HID, cfg.NW, cfg.NP, cfg.NT, cfg.BPC

    xT_e = nc.declare_dram_parameter("xT", [HID, NP], BF16, isOutput=False)
    w1_e = nc.declare_dram_parameter("w1e", [HID, NW], BF16, isOutput=False)
    w2_e = nc.declare_dram_parameter("w2e", [HID, NW], BF16, isOutput=False)
    b1_e = nc.declare_dram_parameter("b1b", [P, HID], F32, isOutput=False)
    b2_e = nc.declare_dram_parameter("b2b", [P, HID], F32, isOutput=False)
    isrc_e = nc.declare_dram_parameter("isrc", [BPC, P, 8 * T], I16, isOutput=False)
    isrc2_e = nc.declare_dram_parameter("isrc2", [BPC, P, 8 * T], I16, isOutput=False)
    iblk_e = nc.declare_dram_parameter("iblk", [BPC, P, 8], I16, isOutput=False)
    iblk2_e = nc.declare_dram_parameter("iblk2", [BPC, P, 8], I16, isOutput=False)
    loc_e = nc.declare_dram_parameter("loce", [BPC, P, T], I16, isOutput=False)
    se_e = nc.declare_dram_parameter("sede", [BPC, P, 2], F32, isOutput=False)
    out_e = nc.declare_dram_parameter("out", [BPC * P, HID], F32, isOutput=True)

    tabf1 = nc.dram_tensor("tabf1", [NP, cfg.FAT], BF16)
    fat2own = nc.dram_tensor("fat2own", [BPC * P, cfg.FAT], BF16)
    tabf2sh = nc.dram_tensor("tabf2sh", [NP, cfg.FAT], BF16, addr_space="Shared")

    with tile.TileContext(nc) as tc:
        with tc.tile_pool(name="const", bufs=1) as cp:
            cn = {}
            for nm, src in (("w1a", w1_e), ("w1b", w1_e), ("w2a", w2_e),
                            ("w2b", w2_e)):
                t = cp.tile([P, NW], BF16, tag=nm)
                lo = 0 if nm.endswith("a") else P
                nc.sync.dma_start(t[:], src[lo : lo + P, :])
                cn[nm] = t
            b1t = cp.tile([P, HID], F32)
            nc.sync.dma_start(b1t[:], b1_e[:, :])
            cn["b1t"] = b1t
            b2t = cp.tile([P, HID], F32)
            nc.sync.dma_start(b2t[:], b2_e[:, :])
            cn["b2t"] = b2t
            idn = cp.tile([P, P], BF16)
            make_identity(nc, idn[:])
            cn["idn"] = idn
            iota_dt = cp.tile([P, P, T], I16)
            nc.gpsimd.iota(iota_dt[:], pattern=[[1, P], [0, T]],
                           channel_multiplier=0)
            cn["iota_dt"] = iota_dt
            iota_e = cp.tile([P, T * P], I16)
            nc.gpsimd.iota(iota_e[:], pattern=[[1, T * P]],
                           channel_multiplier=0)
            cn["iota_e"] = iota_e

            # ---- layer-1 tables (full, redundant per core) ----
            PAN = 10
            with (
                tc.tile_pool(name="s1", bufs=2) as sp,
                tc.tile_pool(name="fp1", bufs=4) as fp,
                tc.tile_pool(name="ps1", bufs=4, space="PSUM") as pp,
            ):
                for pan in range(-(-NT // PAN)):
                    j0, j1 = pan * PAN, min(NT, (pan + 1) * PAN)
                    w = (j1 - j0) * P
                    xp0 = sp.tile([P, PAN * P], BF16, tag="xp0")
                    nc.sync.dma_start(xp0[:, :w], xT_e[0:P, j0 * P : j1 * P])
                    xp1 = sp.tile([P, PAN * P], BF16, tag="xp1")
                    nc.sync.dma_start(xp1[:, :w], xT_e[P : 2 * P, j0 * P : j1 * P])
                    for j in range(j0, j1):
                        o = (j - j0) * P
                        ps = pp.tile([P, NW], F32, tag="tps")
                        nc.tensor.matmul(ps[:], xp0[:, o : o + P], cn["w1a"][:],
                                         start=True, stop=False)
                        nc.tensor.matmul(ps[:], xp1[:, o : o + P], cn["w1b"][:],
                                         start=False, stop=True)
                        fat = fp.tile([P, cfg.FAT], BF16, tag="fat")
                        nc.scalar.activation(fat[:, 0:NW], ps[:], AF.Copy)
                        nc.scalar.dma_start(tabf1[j * P : (j + 1) * P, :], fat[:])

            _edge_phase(nc, tc, cfg, T, cn, 1, tabf1, isrc_e, iblk_e, loc_e,
                        se_e, fat2own=fat2own, tabf2sh=tabf2sh)
            _edge_phase(nc, tc, cfg, T, cn, 2, tabf2sh, isrc2_e, iblk2_e,
                        loc_e, se_e, out_e=out_e)
    nc.finalize()
    return nc


# --------------------------------------------------------------------------
# Entry point
# --------------------------------------------------------------------------
def run_gat(inputs, cfg=None, trace=False):
    cfg = cfg or Cfg()
    in_maps, T = preprocess(cfg, **inputs)
    nc = build_program(cfg, T)
    res = run_bass_kernel_spmd(nc, in_maps, list(range(cfg.NC)), trace=trace)
    out = np.concatenate([res.results[c]["out"] for c in range(cfg.NC)], axis=0)
    return out[: cfg.N], res


def kernel(**inputs) -> np.ndarray:
    out, _ = run_gat(inputs)
    return np.ascontiguousarray(out, dtype=np.float32)
